# revision 1
# baseline (speedup 1.0000x reference)
import sys

sys.path.insert(0, "/opt/trn_rl_repo")

import numpy as np

import concourse.bass as bass
import concourse.mybir as mybir
import concourse.tile as tile
from concourse import bacc
from concourse.bass_utils import run_bass_kernel_spmd

F32 = mybir.dt.float32
F32R = mybir.dt.float32r
BF16 = mybir.dt.bfloat16
AX = mybir.AxisListType
OP = mybir.AluOpType
AF = mybir.ActivationFunctionType

B, NL, NH, DL, DH = 64, 512, 32, 16, 32
NCORES = 8
ILOC = NL // NCORES
NG = ILOC // 8
NCB = B // 16
KL = NH * DH
ATT = 5.656854249492381
EPS = 1e-20
KREP = 32

_CACHE = {}


def _bcast(ap, n):
    return bass.AP(ap.tensor, ap.offset, list(ap.ap) + [[0, n]])


def _r(ap):
    return ap.bitcast(F32R)


def build_program(krep=KREP):
    nc = bacc.Bacc(
        "TRN2",
        target_bir_lowering=False,
        debug=False,
        enable_asserts=False,
        num_devices=NCORES,
    )

    WUL = nc.dram_tensor("WUL", [NG, 128, 1600], F32, kind="ExternalInput")
    Ones = nc.dram_tensor("Ones", [NCB, 128, B], BF16, kind="ExternalInput")
    Bcast = nc.dram_tensor("Bcast", [NCB, B, 128], F32, kind="ExternalInput")
    out_sh = nc.dram_tensor("out_sh", [B // NCORES, KL], F32, kind="ExternalOutput")

    S_part = [nc.dram_tensor(f"S_part{i}", [B, KL], F32) for i in range(2)]
    S_full = [
        nc.dram_tensor(f"S_full{i}", [B, KL], F32, addr_space="Shared")
        for i in range(2)
    ]
    Uh_part = [nc.dram_tensor(f"Uh_part{i}", [B, KL], F32) for i in range(2)]
    Uh_my = [
        nc.dram_tensor(f"Uh_my{i}", [B // NCORES, KL], F32) for i in range(2)
    ]

    rg = [list(range(NCORES))]

    from contextlib import ExitStack

    with tile.TileContext(nc) as tc, ExitStack() as ctx:
        persist = ctx.enter_context(tc.tile_pool(name="persist", bufs=1))
        uhat = persist.tile([128, NG, NCB, KL], F32)
        srep = persist.tile([128, NCB, KL], F32)
        asum = persist.tile([128, NCB, NG, 32], F32)
        cc = persist.tile([128, NCB, NG, 32], F32)
        ones_sb = persist.tile([128, NCB, B], BF16)
        bc_sb = persist.tile([B, NCB, 128], F32)
        one_t = persist.tile([128, 1], F32)
        ps_uh_pool = ctx.enter_context(
            tc.tile_pool(name="psuh", bufs=1, space="PSUM")
        )

        nc.sync.dma_start(
            out=ones_sb[:],
            in_=Ones[:, :, :].rearrange("c p b -> p c b"),
        )
        nc.sync.dma_start(
            out=bc_sb[:], in_=Bcast[:, :, :].rearrange("c b m -> b c m")
        )
        nc.vector.memset(one_t[:], 1.0)

        ps_uh = ps_uh_pool.tile([B, KL], F32)
        nc.tensor.matmul(
            ps_uh[0:1, 0:1],
            lhsT=ones_sb[:, 0, 0:1],
            rhs=ones_sb[:, 0, 0:1],
            start=True,
            stop=True,
        )

        pools = {
            "small": ctx.enter_context(tc.tile_pool(name="small", bufs=1)),
            "wul": ctx.enter_context(tc.tile_pool(name="wul", bufs=2)),
            "wul2": ctx.enter_context(tc.tile_pool(name="wul2", bufs=3)),
            "psu": ctx.enter_context(
                tc.tile_pool(name="psu", bufs=2, space="PSUM")
            ),
            "pss": ctx.enter_context(
                tc.tile_pool(name="pss", bufs=1, space="PSUM")
            ),
            "tmp": ctx.enter_context(tc.tile_pool(name="tmp", bufs=3)),
        }

        for rep in range(krep):
            pb = rep % 2
            _body_once(
                nc, tc, pools, uhat, srep, asum, cc, ones_sb, bc_sb, one_t,
                ps_uh, WUL, out_sh,
                S_part[pb], S_full[pb], Uh_part[pb], Uh_my[pb], rg,
            )

    nc.finalize()
    return nc


def _body_once(
    nc, tc, pools, uhat, srep, asum, cc, ones_sb, bc_sb, one_t, ps_uh,
    WUL, out_sh, S_part, S_full, Uh_part, Uh_my, rg,
):
    small = pools["small"]
    wul_pool = pools["wul"]
    wul2_pool = pools["wul2"]
    psu_pool = pools["psu"]
    pss_pool = pools["pss"]
    tmp_pool = pools["tmp"]

    ps_s = pss_pool.tile([B, KL], F32, tag="pss")
    for g in range(NG):
        wul_g = wul_pool.tile([128, KL + B], F32, tag="wul")
        nc.sync.dma_start(out=wul_g[:], in_=WUL[g, :, 0 : KL + B])
        wr_g = wul_g[:, 0:KL]
        ult_g = wul_g[:, KL : KL + B]
        for nch in range(2):
            nc.tensor.matmul(
                ps_s[:, nch * 512 : (nch + 1) * 512],
                lhsT=ult_g,
                rhs=wr_g[:, nch * 512 : (nch + 1) * 512],
                start=(g == 0),
                stop=(g == NG - 1),
            )
    s_sb = small.tile([B, KL], F32, tag="stage4k")
    nc.scalar.copy(out=s_sb[:], in_=ps_s[:])
    nc.sync.dma_start(out=S_part[:, :], in_=s_sb[:])
    nc.gpsimd.collective_compute(
        "AllReduce",
        OP.add,
        replica_groups=rg,
        ins=[S_part[:, :]],
        outs=[S_full[:, :]],
    )

    for g in range(NG):
        wul_g = wul2_pool.tile([128, 1600], F32, tag="wul2")
        nc.sync.dma_start(out=wul_g[:], in_=WUL[g])
        wr_g = wul_g[:, 0:KL]
        for cb in range(NCB):
            lb = wul_g[:, KL + B + 128 * cb : KL + B + 128 * (cb + 1)]
            psu = psu_pool.tile([128, KL], F32, tag="psu")
            for nch in range(2):
                nc.tensor.matmul(
                    psu[:, nch * 512 : (nch + 1) * 512],
                    lhsT=lb,
                    rhs=wr_g[:, nch * 512 : (nch + 1) * 512],
                    start=True,
                    stop=True,
                )
            nc.scalar.copy(out=uhat[:, g, cb, :], in_=psu[:])

    sf_sb = small.tile([B, KL], F32, tag="sf_sb")
    nc.sync.dma_start(out=sf_sb[:], in_=S_full[:, :])
    for cb in range(NCB):
        ps_r = psu_pool.tile([128, KL], F32, tag="psu")
        for nch in range(2):
            nc.tensor.matmul(
                ps_r[:, nch * 512 : (nch + 1) * 512],
                lhsT=bc_sb[:, cb, :],
                rhs=sf_sb[:, nch * 512 : (nch + 1) * 512],
                start=True,
                stop=True,
            )
        nc.scalar.copy(out=srep[:, cb, :], in_=ps_r[:])

    for cb in range(NCB):
        for g in range(NG):
            tmp = tmp_pool.tile([128, 32, 32], F32, tag="tmpB", bufs=2)
            eng = nc.gpsimd if (g % 2 == 0) else nc.vector
            eng.tensor_tensor(
                tmp[:],
                uhat[:, g, cb, :].rearrange("p (k l) -> p k l", l=32),
                srep[:, cb, :].rearrange("p (k l) -> p k l", l=32),
                OP.mult,
            )
            nc.vector.tensor_reduce(
                asum[:, cb, g, :], tmp[:], axis=AX.X, op=OP.add
            )
        mx = small.tile([128, NG], F32, tag="mx")
        nc.vector.tensor_reduce(mx[:], asum[:, cb], axis=AX.X, op=OP.max)
        zs = small.tile([128, NG, 32], F32, tag="zs")
        nc.vector.tensor_tensor(
            zs[:], asum[:, cb], _bcast(mx[:], 32), OP.subtract
        )
        ex = small.tile([128, NG, 32], F32, tag="ex")
        nc.scalar.activation(ex[:], zs[:], AF.Exp, scale=1.0 / ATT)
        sm = small.tile([128, NG], F32, tag="sm")
        nc.vector.tensor_reduce(sm[:], ex[:], axis=AX.X, op=OP.add)
        rc = small.tile([128, NG], F32, tag="rc")
        nc.vector.reciprocal(rc[:], sm[:])
        nc.vector.tensor_tensor(cc[:, cb], ex[:], _bcast(rc[:], 32), OP.mult)

        for g in range(NG):
            tmp2 = tmp_pool.tile([128, 32, 32], BF16, tag="tmpC")
            eng = nc.gpsimd if (g % 2 == 1) else nc.vector
            eng.tensor_tensor(
                tmp2[:],
                uhat[:, g, cb, :].rearrange("p (k l) -> p k l", l=32),
                _bcast(cc[:, cb, g, :], 32),
                OP.mult,
            )
            for nch in range(2):
                nc.tensor.matmul(
                    ps_uh[:, nch * 512 : (nch + 1) * 512],
                    lhsT=ones_sb[:, cb, :],
                    rhs=tmp2[:].rearrange("p a b -> p (a b)")[
                        :, nch * 512 : (nch + 1) * 512
                    ],
                    start=(g == 0 and cb == 0),
                    stop=(g == NG - 1 and cb == NCB - 1),
                )

    uh_sb = small.tile([B, KL], F32, tag="stage4k")
    nc.scalar.copy(out=uh_sb[:], in_=ps_uh[:])
    nc.sync.dma_start(out=Uh_part[:, :], in_=uh_sb[:])

    nc.gpsimd.collective_compute(
        "ReduceScatter",
        OP.add,
        replica_groups=rg,
        ins=[Uh_part[:, :]],
        outs=[Uh_my[:, :]],
    )
    for h in range(2):
        um = small.tile([128, DH], F32, tag=f"um{h}")
        nc.sync.dma_start(
            out=um[:],
            in_=Uh_my[:, :].rearrange("b (k l) -> (b k) l", l=DH)[
                128 * h : 128 * (h + 1), :
            ],
        )
        sq = small.tile([128, DH], F32, tag=f"sq{h}")
        nc.vector.tensor_tensor(sq[:], um[:], um[:], OP.mult)
        n2 = small.tile([128, 1], F32, tag=f"n2{h}")
        nc.vector.tensor_reduce(n2[:], sq[:], axis=AX.X, op=OP.add)
        nrm = small.tile([128, 1], F32, tag=f"nrm{h}")
        nc.scalar.activation(nrm[:], n2[:], AF.Sqrt)
        en = small.tile([128, 1], F32, tag=f"en{h}")
        nc.scalar.activation(en[:], nrm[:], AF.Exp, scale=-1.0)
        f1 = small.tile([128, 1], F32, tag=f"f1{h}")
        nc.vector.tensor_tensor(f1[:], one_t[:], en[:], OP.subtract)
        nd = small.tile([128, 1], F32, tag=f"nd{h}")
        nc.vector.tensor_scalar_add(nd[:], nrm[:], EPS)
        rn = small.tile([128, 1], F32, tag=f"rn{h}")
        nc.vector.reciprocal(rn[:], nd[:])
        fac = small.tile([128, 1], F32, tag=f"fac{h}")
        nc.vector.tensor_tensor(fac[:], f1[:], rn[:], OP.mult)
        ov = small.tile([128, DH], F32, tag=f"sq{h}")
        nc.vector.tensor_tensor(ov[:], um[:], _bcast(fac[:, 0], DH), OP.mult)
        nc.sync.dma_start(
            out=out_sh[:, :].rearrange("b (k l) -> (b k) l", l=DH)[
                128 * h : 128 * (h + 1), :
            ],
            in_=ov[:],
        )


def host_prep(U_l, W):
    U_l = np.asarray(U_l, dtype=np.float32)
    W = np.asarray(W, dtype=np.float32)
    import ml_dtypes

    ones = np.zeros((NCB, 128, B), dtype=ml_dtypes.bfloat16)
    for cb in range(NCB):
        for i_sub in range(8):
            ones[cb, 16 * i_sub : 16 * (i_sub + 1), 16 * cb : 16 * (cb + 1)] = np.eye(
                16, dtype=ml_dtypes.bfloat16
            )
    bcast = np.zeros((NCB, B, 128), dtype=np.float32)
    for cb in range(NCB):
        for i_sub in range(8):
            bcast[cb, 16 * cb : 16 * (cb + 1), 16 * i_sub : 16 * (i_sub + 1)] = np.eye(
                16, dtype=np.float32
            )
    in_maps = []
    for c in range(NCORES):
        i0 = c * ILOC
        Wsh = W[i0 : i0 + ILOC]
        Wr = np.ascontiguousarray(
            Wsh.reshape(NG, 8, NH, DL, DH).transpose(0, 1, 3, 2, 4)
        ).reshape(NG, 128, KL)
        Ush = U_l[:, i0 : i0 + ILOC, :]
        UlT = np.ascontiguousarray(
            Ush.reshape(B, NG, 8, DL).transpose(1, 2, 3, 0)
        ).reshape(NG, 128, B)
        Lb = np.zeros((NG, NCB, 128, 128), dtype=np.float32)
        blocks = UlT.reshape(NG, 8, DL, NCB, 16)
        for i_sub in range(8):
            Lb[:, :, 16 * i_sub : 16 * i_sub + DL, 16 * i_sub : 16 * (i_sub + 1)] = (
                blocks[:, i_sub].transpose(0, 2, 1, 3)
            )
        WUL = np.concatenate(
            [Wr, UlT, Lb.transpose(0, 2, 1, 3).reshape(NG, 128, NCB * 128)],
            axis=2,
        )
        in_maps.append({"WUL": WUL, "Ones": ones, "Bcast": bcast})
    return in_maps


def _build_executable(nc):
    import jax
    from jax.sharding import Mesh, PartitionSpec
    from jax.experimental.shard_map import shard_map
    from concourse import bass2jax

    bass2jax.install_neuronx_cc_hook()
    partition_name = nc.partition_id_tensor.name if nc.partition_id_tensor else None
    in_names, in_shapes, out_names, out_avals, zero_outs = [], [], [], [], []
    for alloc in nc.m.functions[0].allocations:
        if not isinstance(alloc, mybir.MemoryLocationSet):
            continue
        name = alloc.memorylocations[0].name
        if alloc.kind == "ExternalInput":
            if name != partition_name:
                in_names.append(name)
                in_shapes.append(
                    (tuple(alloc.tensor_shape), mybir.dt.np(alloc.dtype))
                )
        elif alloc.kind == "ExternalOutput":
            shape = tuple(alloc.tensor_shape)
            dtype = mybir.dt.np(alloc.dtype)
            out_names.append(name)
            out_avals.append(jax.core.ShapedArray(shape, dtype))
            zero_outs.append(np.zeros(shape, dtype))
    n_params = len(in_names)
    n_outs = len(out_avals)
    all_names = list(in_names) + out_names
    if partition_name is not None:
        all_names.append(partition_name)

    def _body(*args):
        operands = list(args)
        if partition_name is not None:
            operands.append(bass2jax.partition_id_tensor())
        outs = bass2jax._bass_exec_p.bind(
            *operands,
            out_avals=tuple(out_avals),
            in_names=tuple(all_names),
            out_names=tuple(out_names),
            lowering_input_output_aliases=(),
            sim_require_finite=True,
            sim_require_nnan=True,
            nc=nc,
        )
        return tuple(outs)

    devices = jax.devices()[:NCORES]
    mesh = Mesh(np.asarray(devices), ("core",))
    sharding = jax.sharding.NamedSharding(mesh, PartitionSpec("core"))
    global_avals = [
        jax.ShapeDtypeStruct((NCORES * s[0], *s[1:]), dt, sharding=sharding)
        for (s, dt) in in_shapes
    ] + [
        jax.ShapeDtypeStruct(
            (NCORES * a.shape[0], *a.shape[1:]), a.dtype, sharding=sharding
        )
        for a in out_avals
    ]
    fn = bass2jax.fast_dispatch_compile(
        lambda: jax.jit(
            shard_map(
                _body,
                mesh=mesh,
                in_specs=(PartitionSpec("core"),) * (n_params + n_outs),
                out_specs=(PartitionSpec("core"),) * len(out_names),
                check_rep=False,
            ),
            donate_argnums=tuple(range(n_params, n_params + n_outs)),
            keep_unused=True,
        )
        .lower(*global_avals)
        .compile()
    )

    def stage(in_maps):
        concat_in = [
            np.concatenate(
                [np.asarray(in_maps[c][nm]) for c in range(NCORES)], axis=0
            )
            for nm in in_names
        ]
        zeros = [
            np.zeros((NCORES * z.shape[0], *z.shape[1:]), z.dtype)
            for z in zero_outs
        ]
        return concat_in, zeros

    def run(in_maps):
        import jax as _jax

        concat_in, zeros = stage(in_maps)
        concat_in = [_jax.device_put(a, sharding) for a in concat_in]
        zeros = [_jax.device_put(z, sharding) for z in zeros]
        out_arrs = fn(*concat_in, *zeros)
        out_arrs = [np.asarray(a) for a in _jax.block_until_ready(out_arrs)]
        return [
            {
                nm: out_arrs[i].reshape(NCORES, *out_avals[i].shape)[c]
                for i, nm in enumerate(out_names)
            }
            for c in range(NCORES)
        ]

    run.fn = fn
    run.stage = stage
    run.mesh = mesh
    run.sharding = sharding
    run.out_avals = out_avals
    run.out_names = out_names
    return run


def kernel(U_l, W):
    if "run" not in _CACHE:
        nc = build_program()
        _CACHE["nc"] = nc
        _CACHE["run"] = _build_executable(nc)
    in_maps = host_prep(U_l, W)
    results = _CACHE["run"](in_maps)
    out = np.concatenate(
        [results[c]["out_sh"].reshape(B // NCORES, NH, DH) for c in range(NCORES)],
        axis=0,
    )
    return out



# revision 32
# speedup vs baseline: 1.4370x; 1.4370x over previous
import sys

sys.path.insert(0, "/opt/trn_rl_repo")

import numpy as np

import concourse.bass as bass
import concourse.mybir as mybir
import concourse.tile as tile
from concourse import bacc
from concourse.bass_utils import run_bass_kernel_spmd

F32 = mybir.dt.float32
F32R = mybir.dt.float32r
BF16 = mybir.dt.bfloat16
AX = mybir.AxisListType
OP = mybir.AluOpType
AF = mybir.ActivationFunctionType

B, NL, NH, DL, DH = 64, 512, 32, 16, 32
NCORES = 8
ILOC = NL // NCORES
NG = ILOC // 8
NCB = B // 16
KL = NH * DH
ATT = 5.656854249492381
EPS = 1e-20
KREP = 32

_CACHE = {}


def _bcast(ap, n):
    return bass.AP(ap.tensor, ap.offset, list(ap.ap) + [[0, n]])


def _r(ap):
    return ap.bitcast(F32R)


def build_program_v2(
    krep=KREP,
    eng_b="pdpdpdpd",
    eng_r="dpdpdpdp",
    eng_c="pdpdpdpd",
    skip_ar=False,
    skip_rs=False,
):
    nc = bacc.Bacc(
        "TRN2",
        target_bir_lowering=False,
        debug=False,
        enable_asserts=False,
        num_devices=NCORES,
    )

    WUL = nc.dram_tensor("WUL", [NG, 128, 1600], F32, kind="ExternalInput")
    Ones = nc.dram_tensor("Ones", [NCB, 128, B], BF16, kind="ExternalInput")
    Bcast = nc.dram_tensor("Bcast", [NCB, B, 128], F32, kind="ExternalInput")
    out_sh = nc.dram_tensor("out_sh", [B // NCORES, KL], F32, kind="ExternalOutput")

    S_part = [nc.dram_tensor(f"S_part{i}", [B, KL], F32) for i in range(2)]
    S_full = [
        nc.dram_tensor(f"S_full{i}", [B, KL], F32, addr_space="Shared")
        for i in range(2)
    ]
    Uh_part = [nc.dram_tensor(f"Uh_part{i}", [B, KL], F32) for i in range(2)]
    Uh_my = [
        nc.dram_tensor(f"Uh_my{i}", [B // NCORES, KL], F32) for i in range(2)
    ]

    rg = [list(range(NCORES))]
    ENG = {"d": None, "p": None}

    from contextlib import ExitStack

    with tile.TileContext(nc) as tc, ExitStack() as ctx:
        ENG = {"d": nc.vector, "p": nc.gpsimd}
        persist = ctx.enter_context(tc.tile_pool(name="persist", bufs=1))
        wulr = persist.tile([128, NG, 1600], F32)
        wulr_r = persist.tile([128, NG, 1600], F32R)
        srep = persist.tile([128, NCB, KL], F32)
        asum = persist.tile([128, NCB, NG, 32], F32)
        cc = persist.tile([128, NCB, NG, 32], F32)
        ones_sb = persist.tile([128, NCB, B], BF16)
        bc_sb = persist.tile([B, NCB, 128], F32)
        bc_r = persist.tile([B, NCB, 128], F32R)
        one_t = persist.tile([128, 1], F32)
        ps_uh_pool = ctx.enter_context(
            tc.tile_pool(name="psuh", bufs=1, space="PSUM")
        )

        nc.sync.dma_start(
            out=wulr[:], in_=WUL[:, :, :].rearrange("g p x -> p g x")
        )
        nc.scalar.copy(out=wulr_r[:], in_=wulr[:])
        nc.sync.dma_start(
            out=ones_sb[:],
            in_=Ones[:, :, :].rearrange("c p b -> p c b"),
        )
        nc.sync.dma_start(
            out=bc_sb[:], in_=Bcast[:, :, :].rearrange("c b m -> b c m")
        )
        nc.scalar.copy(out=bc_r[:], in_=bc_sb[:])
        nc.vector.memset(one_t[:], 1.0)

        ps_uh = ps_uh_pool.tile([B, KL], F32)
        nc.tensor.matmul(
            ps_uh[0:1, 0:1],
            lhsT=ones_sb[:, 0, 0:1],
            rhs=ones_sb[:, 0, 0:1],
            start=True,
            stop=True,
        )

        pools = {
            "small": ctx.enter_context(tc.tile_pool(name="small", bufs=1)),
            "psu": ctx.enter_context(
                tc.tile_pool(name="psu", bufs=3, space="PSUM")
            ),
            "tmp": ctx.enter_context(tc.tile_pool(name="tmp", bufs=3)),
        }

        for rep in range(krep):
            pb = rep % 2
            _body_v2(
                nc, tc, pools, wulr_r, srep, asum, cc, ones_sb, bc_sb, one_t,
                ps_uh, out_sh,
                S_part[pb], S_full[pb], Uh_part[pb], Uh_my[pb], rg,
                ENG, eng_b, eng_r, eng_c, skip_ar, skip_rs,
            )

    nc.finalize()
    return nc


def _body_v2(
    nc, tc, pools, wulr, srep, asum, cc, ones_sb, bc_sb, one_t, ps_uh,
    out_sh, S_part, S_full, Uh_part, Uh_my, rg,
    ENG, eng_b, eng_r, eng_c, skip_ar, skip_rs,
):
    small = pools["small"]
    psu_pool = pools["psu"]
    tmp_pool = pools["tmp"]

    def wr(g):
        return wulr[:, g, 0:KL]

    def ult(g):
        return wulr[:, g, KL : KL + B]

    def lb(g, cb):
        return wulr[:, g, KL + B + 128 * cb : KL + B + 128 * (cb + 1)]

    ps_s = psu_pool.tile([128, KL], F32, tag="psu")
    for g in range(NG):
        for nch in range(2):
            nc.tensor.matmul(
                ps_s[0:B, nch * 512 : (nch + 1) * 512],
                lhsT=ult(g),
                rhs=wr(g)[:, nch * 512 : (nch + 1) * 512],
                start=(g == 0),
                stop=(g == NG - 1),
            )
    s_sb = small.tile([B, KL], F32, tag="stage4k")
    nc.scalar.copy(out=s_sb[:], in_=ps_s[0:B, :])
    nc.sync.dma_start(out=S_part[:, :], in_=s_sb[:])
    if skip_ar:
        nc.sync.dma_start(out=S_full[:, :], in_=S_part[:, :])
    else:
        nc.gpsimd.collective_compute(
            "AllReduce",
            OP.add,
            replica_groups=rg,
            ins=[S_part[:, :]],
            outs=[S_full[:, :]],
        )

    sf_sb = small.tile([B, KL], F32, tag="sf_sb")
    nc.sync.dma_start(out=sf_sb[:], in_=S_full[:, :])
    sf_r = small.tile([B, KL], F32R, tag="sf_r")
    nc.scalar.copy(out=sf_r[:], in_=sf_sb[:])
    for cb in range(NCB):
        ps_r = psu_pool.tile([128, KL], F32, tag="psu")
        for nch in range(2):
            nc.tensor.matmul(
                ps_r[:, nch * 512 : (nch + 1) * 512],
                lhsT=bc_sb[:, cb, :],
                rhs=sf_r[:, nch * 512 : (nch + 1) * 512],
                start=True,
                stop=True,
            )
        nc.scalar.copy(out=srep[:, cb, :], in_=ps_r[:])

    for cb in range(NCB):
        for g in range(NG):
            psu = psu_pool.tile([128, KL], F32, tag="psu")
            for nch in range(2):
                nc.tensor.matmul(
                    psu[:, nch * 512 : (nch + 1) * 512],
                    lhsT=lb(g, cb),
                    rhs=wr(g)[:, nch * 512 : (nch + 1) * 512],
                    start=True,
                    stop=True,
                )
            tmp = tmp_pool.tile([128, 32, 32], F32, tag="tmpB")
            ENG[eng_b[g]].tensor_tensor(
                tmp[:],
                psu[:].rearrange("p (k l) -> p k l", l=32),
                srep[:, cb, :].rearrange("p (k l) -> p k l", l=32),
                OP.mult,
            )
            nc.vector.tensor_reduce(
                asum[:, cb, g, :], tmp[:], axis=AX.X, op=OP.add
            )
        mx = small.tile([128, NG], F32, tag="mx")
        nc.vector.tensor_reduce(mx[:], asum[:, cb], axis=AX.X, op=OP.max)
        zs = small.tile([128, NG, 32], F32, tag="zs")
        nc.vector.tensor_tensor(
            zs[:], asum[:, cb], _bcast(mx[:], 32), OP.subtract
        )
        ex = small.tile([128, NG, 32], F32, tag="ex")
        nc.scalar.activation(ex[:], zs[:], AF.Exp, scale=1.0 / ATT)
        sm = small.tile([128, NG], F32, tag="sm")
        nc.vector.tensor_reduce(sm[:], ex[:], axis=AX.X, op=OP.add)
        rc = small.tile([128, NG], F32, tag="rc")
        nc.vector.reciprocal(rc[:], sm[:])
        nc.vector.tensor_tensor(cc[:, cb], ex[:], _bcast(rc[:], 32), OP.mult)

        for g in range(NG):
            psu2 = psu_pool.tile([128, KL], F32, tag="psu")
            for nch in range(2):
                nc.tensor.matmul(
                    psu2[:, nch * 512 : (nch + 1) * 512],
                    lhsT=lb(g, cb),
                    rhs=wr(g)[:, nch * 512 : (nch + 1) * 512],
                    start=True,
                    stop=True,
                )
            tmp2 = tmp_pool.tile([128, 32, 32], BF16, tag="tmpC")
            ENG[eng_c[g]].tensor_tensor(
                tmp2[:],
                psu2[:].rearrange("p (k l) -> p k l", l=32),
                _bcast(cc[:, cb, g, :], 32),
                OP.mult,
            )
            for nch in range(2):
                nc.tensor.matmul(
                    ps_uh[:, nch * 512 : (nch + 1) * 512],
                    lhsT=ones_sb[:, cb, :],
                    rhs=tmp2[:].rearrange("p a b -> p (a b)")[
                        :, nch * 512 : (nch + 1) * 512
                    ],
                    start=(g == 0 and cb == 0),
                    stop=(g == NG - 1 and cb == NCB - 1),
                )

    uh_sb = small.tile([B, KL], F32, tag="stage4k")
    nc.scalar.copy(out=uh_sb[:], in_=ps_uh[:])
    nc.sync.dma_start(out=Uh_part[:, :], in_=uh_sb[:])

    if skip_rs:
        nc.sync.dma_start(out=Uh_my[:, :], in_=Uh_part[0 : B // NCORES, :])
    else:
        nc.gpsimd.collective_compute(
            "ReduceScatter",
            OP.add,
            replica_groups=rg,
            ins=[Uh_part[:, :]],
            outs=[Uh_my[:, :]],
        )
    for h in range(2):
        um = small.tile([128, DH], F32, tag=f"um{h}")
        nc.sync.dma_start(
            out=um[:],
            in_=Uh_my[:, :].rearrange("b (k l) -> (b k) l", l=DH)[
                128 * h : 128 * (h + 1), :
            ],
        )
        sq = small.tile([128, DH], F32, tag=f"sq{h}")
        nc.vector.tensor_tensor(sq[:], um[:], um[:], OP.mult)
        n2 = small.tile([128, 1], F32, tag=f"n2{h}")
        nc.vector.tensor_reduce(n2[:], sq[:], axis=AX.X, op=OP.add)
        nrm = small.tile([128, 1], F32, tag=f"nrm{h}")
        nc.scalar.activation(nrm[:], n2[:], AF.Sqrt)
        en = small.tile([128, 1], F32, tag=f"en{h}")
        nc.scalar.activation(en[:], nrm[:], AF.Exp, scale=-1.0)
        f1 = small.tile([128, 1], F32, tag=f"f1{h}")
        nc.vector.tensor_tensor(f1[:], one_t[:], en[:], OP.subtract)
        nd = small.tile([128, 1], F32, tag=f"nd{h}")
        nc.vector.tensor_scalar_add(nd[:], nrm[:], EPS)
        rn = small.tile([128, 1], F32, tag=f"rn{h}")
        nc.vector.reciprocal(rn[:], nd[:])
        fac = small.tile([128, 1], F32, tag=f"fac{h}")
        nc.vector.tensor_tensor(fac[:], f1[:], rn[:], OP.mult)
        ov = small.tile([128, DH], F32, tag=f"sq{h}")
        nc.vector.tensor_tensor(ov[:], um[:], _bcast(fac[:, 0], DH), OP.mult)
        nc.sync.dma_start(
            out=out_sh[:, :].rearrange("b (k l) -> (b k) l", l=DH)[
                128 * h : 128 * (h + 1), :
            ],
            in_=ov[:],
        )


def _bcast_outer(ap, n):
    return bass.AP(ap.tensor, ap.offset, [list(ap.ap[0]), [0, n]] + [list(x) for x in ap.ap[1:]])


def build_program_v3(
    krep=KREP,
    eng_b="dppdppdp",
    eng_c="dppdppdp",
    uh_dt="f32",
    tmp_dt="f32",
    srep_dt="f32",
    cc_dt="bf16",
    coll="p",
    skip_ar=False,
    skip_rs=False,
):
    DT = {"f32": F32, "bf16": BF16}
    nc = bacc.Bacc(
        "TRN2",
        target_bir_lowering=False,
        debug=False,
        enable_asserts=False,
        num_devices=NCORES,
    )

    WUL = nc.dram_tensor("WUL", [NG, 128, 1600], F32, kind="ExternalInput")
    Ones = nc.dram_tensor("Ones", [NCB, 128, B], BF16, kind="ExternalInput")
    Bcast = nc.dram_tensor("Bcast", [NCB, B, 128], F32, kind="ExternalInput")
    out_sh = nc.dram_tensor("out_sh", [B // NCORES, KL], F32, kind="ExternalOutput")

    S_part = [nc.dram_tensor(f"S_part{i}", [B, KL], F32) for i in range(2)]
    S_full = [
        nc.dram_tensor(f"S_full{i}", [B, KL], F32, addr_space="Shared")
        for i in range(2)
    ]
    Uh_part = [nc.dram_tensor(f"Uh_part{i}", [B, KL], F32) for i in range(2)]
    Uh_my = [
        nc.dram_tensor(f"Uh_my{i}", [B // NCORES, KL], F32) for i in range(2)
    ]

    rg = [list(range(NCORES))]

    from contextlib import ExitStack

    with tile.TileContext(nc) as tc, ExitStack() as ctx:
        ENG = {"d": nc.vector, "p": nc.gpsimd}
        persist = ctx.enter_context(tc.tile_pool(name="persist", bufs=1))
        wulr_r = persist.tile([128, NG, 1600], F32R)
        srep = persist.tile([128, NCB, KL], DT[srep_dt])
        asum = persist.tile([128, NCB, NG, 32], F32)
        cc = persist.tile([128, NCB, NG, 32], DT[cc_dt])
        ones_sb = persist.tile([128, NCB, B], BF16)
        bc_sb = persist.tile([B, NCB, 128], F32)
        one_t = persist.tile([128, 1], F32)
        ps_uh_pool = ctx.enter_context(
            tc.tile_pool(name="psuh", bufs=1, space="PSUM")
        )

        with tc.tile_pool(name="wload", bufs=2) as wload:
            for g in range(NG):
                sc = wload.tile([128, 1600], F32, tag="wld")
                nc.sync.dma_start(out=sc[:], in_=WUL[g])
                nc.scalar.copy(out=wulr_r[:, g, :], in_=sc[:])
        nc.sync.dma_start(
            out=bc_sb[:], in_=Bcast[:, :, :].rearrange("c b m -> b c m")
        )
        nc.sync.dma_start(
            out=ones_sb[:],
            in_=Ones[:, :, :].rearrange("c p b -> p c b"),
        )
        nc.vector.memset(one_t[:], 1.0)

        ps_uh = ps_uh_pool.tile([B, KL], F32)
        nc.tensor.matmul(
            ps_uh[0:1, 0:1],
            lhsT=ones_sb[:, 0, 0:1],
            rhs=ones_sb[:, 0, 0:1],
            start=True,
            stop=True,
        )

        pools = {
            "small": ctx.enter_context(tc.tile_pool(name="small", bufs=1)),
            "psu": ctx.enter_context(
                tc.tile_pool(name="psu", bufs=3, space="PSUM")
            ),
            "uh": ctx.enter_context(tc.tile_pool(name="uh", bufs=2)),
            "tmp": ctx.enter_context(tc.tile_pool(name="tmp", bufs=2)),
        }

        env = dict(
            nc=nc, pools=pools, wulr=wulr_r, srep=srep, asum=asum, cc=cc,
            ones_sb=ones_sb, bc_sb=bc_sb, one_t=one_t, ps_uh=ps_uh,
            out_sh=out_sh, S_part=S_part, S_full=S_full, Uh_part=Uh_part,
            Uh_my=Uh_my, rg=rg, ENG=ENG, eng_b=eng_b, eng_c=eng_c,
            UH_DT=DT[uh_dt], TMP_DT=DT[tmp_dt], coll=coll,
            skip_ar=skip_ar, skip_rs=skip_rs, krep=krep,
        )
        _phase_s(env, 0)
        for rep in range(krep):
            _body_v3(env, rep)

    nc.finalize()
    return nc


def _phase_s(env, rep):
    nc = env["nc"]
    wulr = env["wulr"]
    small = env["pools"]["small"]
    psu_pool = env["pools"]["psu"]
    pb = rep % 2
    S_part, S_full = env["S_part"][pb], env["S_full"][pb]

    ps_s = psu_pool.tile([128, KL], F32, tag="psu")
    for g in range(NG):
        wr_g = wulr[:, g, 0:KL]
        ult_g = wulr[:, g, KL : KL + B]
        for nch in range(2):
            nc.tensor.matmul(
                ps_s[0:B, nch * 512 : (nch + 1) * 512],
                lhsT=ult_g,
                rhs=wr_g[:, nch * 512 : (nch + 1) * 512],
                start=(g == 0),
                stop=(g == NG - 1),
            )
    s_sb = small.tile([B, KL], F32, tag="s_sb")
    nc.scalar.copy(out=s_sb[:], in_=ps_s[0:B, :])
    nc.sync.dma_start(out=S_part[:, :], in_=s_sb[:])
    if env["skip_ar"]:
        nc.sync.dma_start(out=S_full[:, :], in_=S_part[:, :])
    else:
        import concourse.bass as _bass

        _bass.BassGpSimd.collective_compute(
            nc.gpsimd, "AllReduce", OP.add, replica_groups=env["rg"],
            ins=[S_part[:, :]], outs=[S_full[:, :]],
        )


def _body_v3(env, rep):
    nc = env["nc"]
    wulr = env["wulr"]
    srep, asum, cc = env["srep"], env["asum"], env["cc"]
    ones_sb, bc_sb, one_t, ps_uh = (
        env["ones_sb"], env["bc_sb"], env["one_t"], env["ps_uh"])
    out_sh = env["out_sh"]
    ENG, eng_b, eng_c = env["ENG"], env["eng_b"], env["eng_c"]
    UH_DT, TMP_DT = env["UH_DT"], env["TMP_DT"]
    small = env["pools"]["small"]
    psu_pool = env["pools"]["psu"]
    uh_pool = env["pools"]["uh"]
    tmp_pool = env["pools"]["tmp"]
    pb = rep % 2
    S_full = env["S_full"][pb]
    Uh_part, Uh_my = env["Uh_part"][pb], env["Uh_my"][pb]
    rg = env["rg"]

    def wr(g):
        return wulr[:, g, 0:KL]

    def lb(g, cb):
        return wulr[:, g, KL + B + 128 * cb : KL + B + 128 * (cb + 1)]

    uh_tiles = {}

    def emit_uhat(cb):
        uh_cb = uh_pool.tile([128, NG, KL], UH_DT, tag="uhcb")
        for g in range(NG):
            psu = psu_pool.tile([128, KL], F32, tag="psu")
            for nch in range(2):
                nc.tensor.matmul(
                    psu[:, nch * 512 : (nch + 1) * 512],
                    lhsT=lb(g, cb),
                    rhs=wr(g)[:, nch * 512 : (nch + 1) * 512],
                    start=True,
                    stop=True,
                )
            nc.scalar.copy(out=uh_cb[:, g, :], in_=psu[:])
        uh_tiles[cb] = uh_cb

    emit_uhat(0)
    emit_uhat(1)

    sf_sb = small.tile([B, KL], F32, tag="sf_sb")
    nc.sync.dma_start(out=sf_sb[:], in_=S_full[:, :])
    for cb in range(NCB):
        ps_r = psu_pool.tile([128, KL], F32, tag="psu")
        for nch in range(2):
            nc.tensor.matmul(
                ps_r[:, nch * 512 : (nch + 1) * 512],
                lhsT=bc_sb[:, cb, :],
                rhs=sf_sb[:, nch * 512 : (nch + 1) * 512],
                start=True,
                stop=True,
            )
        nc.scalar.copy(out=srep[:, cb, :], in_=ps_r[:])

    half = NG // 2
    for cb in range(NCB):
        uh_cb = uh_tiles.pop(cb)
        for h in range(2):
            gs = slice(h * half, (h + 1) * half)
            tmp_h = tmp_pool.tile([128, half, 32, 32], TMP_DT, tag="tmpB")
            ENG[eng_b[cb * 2 + h]].tensor_tensor(
                tmp_h[:],
                uh_cb[:, gs].rearrange("p g (k l) -> p g k l", l=32),
                _bcast_outer(
                    srep[:, cb, :].rearrange("p (k l) -> p k l", l=32), half
                ),
                OP.mult,
            )
            nc.vector.tensor_reduce(
                asum[:, cb, gs], tmp_h[:], axis=AX.X, op=OP.add
            )
        mx = small.tile([128, NG], F32, tag="mx")
        nc.vector.tensor_reduce(mx[:], asum[:, cb], axis=AX.X, op=OP.max)
        zs = small.tile([128, NG, 32], F32, tag="zs")
        nc.vector.tensor_tensor(
            zs[:], asum[:, cb], _bcast(mx[:], 32), OP.subtract
        )
        ex = small.tile([128, NG, 32], F32, tag="ex")
        nc.scalar.activation(ex[:], zs[:], AF.Exp, scale=1.0 / ATT)
        sm = small.tile([128, NG], F32, tag="sm")
        nc.vector.tensor_reduce(sm[:], ex[:], axis=AX.X, op=OP.add)
        rc = small.tile([128, NG], F32, tag="rc")
        nc.vector.reciprocal(rc[:], sm[:])
        nc.vector.tensor_tensor(cc[:, cb], ex[:], _bcast(rc[:], 32), OP.mult)

        for h in range(2):
            gs = slice(h * half, (h + 1) * half)
            tmp2_h = tmp_pool.tile([128, half, 32, 32], BF16, tag="tmpC")
            ENG[eng_c[cb * 2 + h]].tensor_tensor(
                tmp2_h[:],
                uh_cb[:, gs].rearrange("p g (k l) -> p g k l", l=32),
                _bcast(cc[:, cb, gs], 32),
                OP.mult,
            )
            flat2 = tmp2_h[:].rearrange("p g a b -> p (g a b)")
            for nch in range(half * 2):
                nc.tensor.matmul(
                    ps_uh[:, (nch % 2) * 512 : (nch % 2 + 1) * 512],
                    lhsT=ones_sb[:, cb, :],
                    rhs=flat2[:, nch * 512 : (nch + 1) * 512],
                    start=(nch < 2 and h == 0 and cb == 0),
                    stop=(nch >= half * 2 - 2 and h == 1 and cb == NCB - 1),
                )
        if cb + 2 < NCB:
            emit_uhat(cb + 2)

    if rep + 1 < env["krep"]:
        _phase_s(env, rep + 1)

    uh_sb = small.tile([B, KL], F32, tag="stage4k")
    nc.scalar.copy(out=uh_sb[:], in_=ps_uh[:])
    nc.sync.dma_start(out=Uh_part[:, :], in_=uh_sb[:])

    if env["skip_rs"]:
        nc.sync.dma_start(out=Uh_my[:, :], in_=Uh_part[0 : B // NCORES, :])
    else:
        import concourse.bass as _bass

        _bass.BassGpSimd.collective_compute(
            nc.gpsimd, "ReduceScatter", OP.add, replica_groups=rg,
            ins=[Uh_part[:, :]], outs=[Uh_my[:, :]],
        )
    for h in range(2):
        um = small.tile([128, DH], F32, tag=f"um{h}")
        nc.sync.dma_start(
            out=um[:],
            in_=Uh_my[:, :].rearrange("b (k l) -> (b k) l", l=DH)[
                128 * h : 128 * (h + 1), :
            ],
        )
        sq = small.tile([128, DH], F32, tag=f"sq{h}")
        nc.vector.tensor_tensor(sq[:], um[:], um[:], OP.mult)
        n2 = small.tile([128, 1], F32, tag=f"n2{h}")
        nc.vector.tensor_reduce(n2[:], sq[:], axis=AX.X, op=OP.add)
        nrm = small.tile([128, 1], F32, tag=f"nrm{h}")
        nc.scalar.activation(nrm[:], n2[:], AF.Sqrt)
        en = small.tile([128, 1], F32, tag=f"en{h}")
        nc.scalar.activation(en[:], nrm[:], AF.Exp, scale=-1.0)
        f1 = small.tile([128, 1], F32, tag=f"f1{h}")
        nc.vector.tensor_tensor(f1[:], one_t[:], en[:], OP.subtract)
        nd = small.tile([128, 1], F32, tag=f"nd{h}")
        nc.vector.tensor_scalar_add(nd[:], nrm[:], EPS)
        rn = small.tile([128, 1], F32, tag=f"rn{h}")
        nc.vector.reciprocal(rn[:], nd[:])
        fac = small.tile([128, 1], F32, tag=f"fac{h}")
        nc.vector.tensor_tensor(fac[:], f1[:], rn[:], OP.mult)
        ov = small.tile([128, DH], F32, tag=f"sq{h}")
        nc.vector.tensor_tensor(ov[:], um[:], _bcast(fac[:, 0], DH), OP.mult)
        nc.sync.dma_start(
            out=out_sh[:, :].rearrange("b (k l) -> (b k) l", l=DH)[
                128 * h : 128 * (h + 1), :
            ],
            in_=ov[:],
        )


def build_program(krep=KREP, skip_ar=False, skip_rs=False, **probe):
    nc = bacc.Bacc(
        "TRN2",
        target_bir_lowering=False,
        debug=False,
        enable_asserts=False,
        num_devices=NCORES,
    )

    WUL = nc.dram_tensor("WUL", [NG, 128, 1600], F32, kind="ExternalInput")
    Ones = nc.dram_tensor("Ones", [NCB, 128, B], BF16, kind="ExternalInput")
    Bcast = nc.dram_tensor("Bcast", [NCB, B, 128], F32, kind="ExternalInput")
    out_sh = nc.dram_tensor("out_sh", [B // NCORES, KL], F32, kind="ExternalOutput")

    S_part = [nc.dram_tensor(f"S_part{i}", [B, KL], F32) for i in range(2)]
    S_full = [
        nc.dram_tensor(f"S_full{i}", [B, KL], F32, addr_space="Shared")
        for i in range(2)
    ]
    Uh_part = [nc.dram_tensor(f"Uh_part{i}", [B, KL], F32) for i in range(2)]
    Uh_my = [
        nc.dram_tensor(f"Uh_my{i}", [B // NCORES, KL], F32) for i in range(2)
    ]

    rg = [list(range(NCORES))]

    from contextlib import ExitStack

    with tile.TileContext(nc) as tc, ExitStack() as ctx:
        persist = ctx.enter_context(tc.tile_pool(name="persist", bufs=1))
        uhat = persist.tile([128, NG, NCB, KL], F32)
        srep = persist.tile([128, NCB, KL], F32)
        asum = persist.tile([128, NCB, NG, 32], F32)
        cc = persist.tile([128, NCB, NG, 32], F32)
        ones_sb = persist.tile([128, NCB, B], BF16)
        bc_sb = persist.tile([B, NCB, 128], F32)
        one_t = persist.tile([128, 1], F32)
        ps_uh_pool = ctx.enter_context(
            tc.tile_pool(name="psuh", bufs=1, space="PSUM")
        )

        nc.sync.dma_start(
            out=ones_sb[:],
            in_=Ones[:, :, :].rearrange("c p b -> p c b"),
        )
        nc.sync.dma_start(
            out=bc_sb[:], in_=Bcast[:, :, :].rearrange("c b m -> b c m")
        )
        nc.vector.memset(one_t[:], 1.0)

        if probe.get("skip_uhat") or probe.get("skip_acopy"):
            nc.vector.memset(uhat[:], 0.5)
        if probe.get("skip_bmul"):
            nc.vector.memset(asum[:], 0.5)
        dummy2 = None
        if probe.get("skip_cmul"):
            dummy2 = persist.tile([128, 32, 32], BF16)
            nc.vector.memset(dummy2[:], 0.5)
        probe = dict(probe, dummy2=dummy2)

        ps_uh = ps_uh_pool.tile([B, KL], F32)
        nc.tensor.matmul(
            ps_uh[0:1, 0:1],
            lhsT=ones_sb[:, 0, 0:1],
            rhs=ones_sb[:, 0, 0:1],
            start=True,
            stop=True,
        )

        pools = {
            "small": ctx.enter_context(tc.tile_pool(name="small", bufs=1)),
            "wul": ctx.enter_context(tc.tile_pool(name="wul", bufs=2)),
            "wul2": ctx.enter_context(tc.tile_pool(name="wul2", bufs=3)),
            "psu": ctx.enter_context(
                tc.tile_pool(name="psu", bufs=2, space="PSUM")
            ),
            "pss": ctx.enter_context(
                tc.tile_pool(name="pss", bufs=1, space="PSUM")
            ),
            "tmp": ctx.enter_context(tc.tile_pool(name="tmp", bufs=3)),
        }

        for rep in range(krep):
            pb = rep % 2
            _body_once(
                nc, tc, pools, uhat, srep, asum, cc, ones_sb, bc_sb, one_t,
                ps_uh, WUL, out_sh,
                S_part[pb], S_full[pb], Uh_part[pb], Uh_my[pb], rg,
                skip_ar=skip_ar, skip_rs=skip_rs, **probe,
            )

    nc.finalize()
    return nc


def _body_once(
    nc, tc, pools, uhat, srep, asum, cc, ones_sb, bc_sb, one_t, ps_uh,
    WUL, out_sh, S_part, S_full, Uh_part, Uh_my, rg,
    skip_ar=False, skip_rs=False, skip_bmul=False, skip_cmul=False,
    skip_uhat=False, skip_acopy=False, mul_eng=None, dummy2=None,
):
    small = pools["small"]
    wul_pool = pools["wul"]
    wul2_pool = pools["wul2"]
    psu_pool = pools["psu"]
    pss_pool = pools["pss"]
    tmp_pool = pools["tmp"]

    ps_s = pss_pool.tile([B, KL], F32, tag="pss")
    for g in range(NG):
        wul_g = wul_pool.tile([128, KL + B], F32, tag="wul")
        nc.sync.dma_start(out=wul_g[:], in_=WUL[g, :, 0 : KL + B])
        wr_g = wul_g[:, 0:KL]
        ult_g = wul_g[:, KL : KL + B]
        for nch in range(2):
            nc.tensor.matmul(
                ps_s[:, nch * 512 : (nch + 1) * 512],
                lhsT=ult_g,
                rhs=wr_g[:, nch * 512 : (nch + 1) * 512],
                start=(g == 0),
                stop=(g == NG - 1),
            )
    s_sb = small.tile([B, KL], F32, tag="stage4k")
    nc.scalar.copy(out=s_sb[:], in_=ps_s[:])
    nc.sync.dma_start(out=S_part[:, :], in_=s_sb[:])
    if skip_ar:
        nc.sync.dma_start(out=S_full[:, :], in_=S_part[:, :])
    else:
        nc.gpsimd.collective_compute(
            "AllReduce",
            OP.add,
            replica_groups=rg,
            ins=[S_part[:, :]],
            outs=[S_full[:, :]],
        )

    for g in range(NG):
        if skip_uhat:
            break
        wul_g = wul2_pool.tile([128, 1600], F32, tag="wul2")
        nc.sync.dma_start(out=wul_g[:], in_=WUL[g])
        wr_g = wul_g[:, 0:KL]
        for cb in range(NCB):
            lb = wul_g[:, KL + B + 128 * cb : KL + B + 128 * (cb + 1)]
            psu = psu_pool.tile([128, KL], F32, tag="psu")
            for nch in range(2):
                nc.tensor.matmul(
                    psu[:, nch * 512 : (nch + 1) * 512],
                    lhsT=lb,
                    rhs=wr_g[:, nch * 512 : (nch + 1) * 512],
                    start=True,
                    stop=True,
                )
            if not skip_acopy:
                nc.scalar.copy(out=uhat[:, g, cb, :], in_=psu[:])

    sf_sb = small.tile([B, KL], F32, tag="sf_sb")
    nc.sync.dma_start(out=sf_sb[:], in_=S_full[:, :])
    for cb in range(NCB):
        ps_r = psu_pool.tile([128, KL], F32, tag="psu")
        for nch in range(2):
            nc.tensor.matmul(
                ps_r[:, nch * 512 : (nch + 1) * 512],
                lhsT=bc_sb[:, cb, :],
                rhs=sf_sb[:, nch * 512 : (nch + 1) * 512],
                start=True,
                stop=True,
            )
        nc.scalar.copy(out=srep[:, cb, :], in_=ps_r[:])

    for cb in range(NCB):
        for g in range(NG):
            if skip_bmul:
                break
            tmp = tmp_pool.tile([128, 32, 32], F32, tag="tmpB", bufs=2)
            eng = nc.gpsimd if (g % 2 == 0) else nc.vector
            if mul_eng == "dve":
                eng = nc.vector
            elif mul_eng == "gps":
                eng = nc.gpsimd
            eng.tensor_tensor(
                tmp[:],
                uhat[:, g, cb, :].rearrange("p (k l) -> p k l", l=32),
                srep[:, cb, :].rearrange("p (k l) -> p k l", l=32),
                OP.mult,
            )
            nc.vector.tensor_reduce(
                asum[:, cb, g, :], tmp[:], axis=AX.X, op=OP.add
            )
        mx = small.tile([128, NG], F32, tag="mx")
        nc.vector.tensor_reduce(mx[:], asum[:, cb], axis=AX.X, op=OP.max)
        zs = small.tile([128, NG, 32], F32, tag="zs")
        nc.vector.tensor_tensor(
            zs[:], asum[:, cb], _bcast(mx[:], 32), OP.subtract
        )
        ex = small.tile([128, NG, 32], F32, tag="ex")
        nc.scalar.activation(ex[:], zs[:], AF.Exp, scale=1.0 / ATT)
        sm = small.tile([128, NG], F32, tag="sm")
        nc.vector.tensor_reduce(sm[:], ex[:], axis=AX.X, op=OP.add)
        rc = small.tile([128, NG], F32, tag="rc")
        nc.vector.reciprocal(rc[:], sm[:])
        nc.vector.tensor_tensor(cc[:, cb], ex[:], _bcast(rc[:], 32), OP.mult)

        for g in range(NG):
            if skip_cmul:
                tmp2 = dummy2
            else:
                tmp2 = tmp_pool.tile([128, 32, 32], BF16, tag="tmpC")
                eng = nc.gpsimd if (g % 2 == 1) else nc.vector
                if mul_eng == "dve":
                    eng = nc.vector
                elif mul_eng == "gps":
                    eng = nc.gpsimd
                eng.tensor_tensor(
                    tmp2[:],
                    uhat[:, g, cb, :].rearrange("p (k l) -> p k l", l=32),
                    _bcast(cc[:, cb, g, :], 32),
                    OP.mult,
                )
            for nch in range(2):
                nc.tensor.matmul(
                    ps_uh[:, nch * 512 : (nch + 1) * 512],
                    lhsT=ones_sb[:, cb, :],
                    rhs=tmp2[:].rearrange("p a b -> p (a b)")[
                        :, nch * 512 : (nch + 1) * 512
                    ],
                    start=(g == 0 and cb == 0),
                    stop=(g == NG - 1 and cb == NCB - 1),
                )

    uh_sb = small.tile([B, KL], F32, tag="stage4k")
    nc.scalar.copy(out=uh_sb[:], in_=ps_uh[:])
    nc.sync.dma_start(out=Uh_part[:, :], in_=uh_sb[:])

    if skip_rs:
        nc.sync.dma_start(out=Uh_my[:, :], in_=Uh_part[0 : B // NCORES, :])
    else:
        nc.gpsimd.collective_compute(
            "ReduceScatter",
            OP.add,
            replica_groups=rg,
            ins=[Uh_part[:, :]],
            outs=[Uh_my[:, :]],
        )
    for h in range(2):
        um = small.tile([128, DH], F32, tag=f"um{h}")
        nc.sync.dma_start(
            out=um[:],
            in_=Uh_my[:, :].rearrange("b (k l) -> (b k) l", l=DH)[
                128 * h : 128 * (h + 1), :
            ],
        )
        sq = small.tile([128, DH], F32, tag=f"sq{h}")
        nc.vector.tensor_tensor(sq[:], um[:], um[:], OP.mult)
        n2 = small.tile([128, 1], F32, tag=f"n2{h}")
        nc.vector.tensor_reduce(n2[:], sq[:], axis=AX.X, op=OP.add)
        nrm = small.tile([128, 1], F32, tag=f"nrm{h}")
        nc.scalar.activation(nrm[:], n2[:], AF.Sqrt)
        en = small.tile([128, 1], F32, tag=f"en{h}")
        nc.scalar.activation(en[:], nrm[:], AF.Exp, scale=-1.0)
        f1 = small.tile([128, 1], F32, tag=f"f1{h}")
        nc.vector.tensor_tensor(f1[:], one_t[:], en[:], OP.subtract)
        nd = small.tile([128, 1], F32, tag=f"nd{h}")
        nc.vector.tensor_scalar_add(nd[:], nrm[:], EPS)
        rn = small.tile([128, 1], F32, tag=f"rn{h}")
        nc.vector.reciprocal(rn[:], nd[:])
        fac = small.tile([128, 1], F32, tag=f"fac{h}")
        nc.vector.tensor_tensor(fac[:], f1[:], rn[:], OP.mult)
        ov = small.tile([128, DH], F32, tag=f"sq{h}")
        nc.vector.tensor_tensor(ov[:], um[:], _bcast(fac[:, 0], DH), OP.mult)
        nc.sync.dma_start(
            out=out_sh[:, :].rearrange("b (k l) -> (b k) l", l=DH)[
                128 * h : 128 * (h + 1), :
            ],
            in_=ov[:],
        )


def host_prep(U_l, W):
    U_l = np.asarray(U_l, dtype=np.float32)
    W = np.asarray(W, dtype=np.float32)
    import ml_dtypes

    ones = np.zeros((NCB, 128, B), dtype=ml_dtypes.bfloat16)
    for cb in range(NCB):
        for i_sub in range(8):
            ones[cb, 16 * i_sub : 16 * (i_sub + 1), 16 * cb : 16 * (cb + 1)] = np.eye(
                16, dtype=ml_dtypes.bfloat16
            )
    bcast = np.zeros((NCB, B, 128), dtype=np.float32)
    for cb in range(NCB):
        for i_sub in range(8):
            bcast[cb, 16 * cb : 16 * (cb + 1), 16 * i_sub : 16 * (i_sub + 1)] = np.eye(
                16, dtype=np.float32
            )
    in_maps = []
    for c in range(NCORES):
        i0 = c * ILOC
        Wsh = W[i0 : i0 + ILOC]
        Wr = np.ascontiguousarray(
            Wsh.reshape(NG, 8, NH, DL, DH).transpose(0, 1, 3, 2, 4)
        ).reshape(NG, 128, KL)
        Ush = U_l[:, i0 : i0 + ILOC, :]
        UlT = np.ascontiguousarray(
            Ush.reshape(B, NG, 8, DL).transpose(1, 2, 3, 0)
        ).reshape(NG, 128, B)
        Lb = np.zeros((NG, NCB, 128, 128), dtype=np.float32)
        blocks = UlT.reshape(NG, 8, DL, NCB, 16)
        for i_sub in range(8):
            Lb[:, :, 16 * i_sub : 16 * i_sub + DL, 16 * i_sub : 16 * (i_sub + 1)] = (
                blocks[:, i_sub].transpose(0, 2, 1, 3)
            )
        WUL = np.concatenate(
            [Wr, UlT, Lb.transpose(0, 2, 1, 3).reshape(NG, 128, NCB * 128)],
            axis=2,
        )
        in_maps.append({"WUL": WUL, "Ones": ones, "Bcast": bcast})
    return in_maps


def _build_executable(nc):
    import jax
    from jax.sharding import Mesh, PartitionSpec
    from jax.experimental.shard_map import shard_map
    from concourse import bass2jax

    bass2jax.install_neuronx_cc_hook()
    partition_name = nc.partition_id_tensor.name if nc.partition_id_tensor else None
    in_names, in_shapes, out_names, out_avals, zero_outs = [], [], [], [], []
    for alloc in nc.m.functions[0].allocations:
        if not isinstance(alloc, mybir.MemoryLocationSet):
            continue
        name = alloc.memorylocations[0].name
        if alloc.kind == "ExternalInput":
            if name != partition_name:
                in_names.append(name)
                in_shapes.append(
                    (tuple(alloc.tensor_shape), mybir.dt.np(alloc.dtype))
                )
        elif alloc.kind == "ExternalOutput":
            shape = tuple(alloc.tensor_shape)
            dtype = mybir.dt.np(alloc.dtype)
            out_names.append(name)
            out_avals.append(jax.core.ShapedArray(shape, dtype))
            zero_outs.append(np.zeros(shape, dtype))
    n_params = len(in_names)
    n_outs = len(out_avals)
    all_names = list(in_names) + out_names
    if partition_name is not None:
        all_names.append(partition_name)

    def _body(*args):
        operands = list(args)
        if partition_name is not None:
            operands.append(bass2jax.partition_id_tensor())
        outs = bass2jax._bass_exec_p.bind(
            *operands,
            out_avals=tuple(out_avals),
            in_names=tuple(all_names),
            out_names=tuple(out_names),
            lowering_input_output_aliases=(),
            sim_require_finite=True,
            sim_require_nnan=True,
            nc=nc,
        )
        return tuple(outs)

    devices = jax.devices()[:NCORES]
    mesh = Mesh(np.asarray(devices), ("core",))
    sharding = jax.sharding.NamedSharding(mesh, PartitionSpec("core"))
    global_avals = [
        jax.ShapeDtypeStruct((NCORES * s[0], *s[1:]), dt, sharding=sharding)
        for (s, dt) in in_shapes
    ] + [
        jax.ShapeDtypeStruct(
            (NCORES * a.shape[0], *a.shape[1:]), a.dtype, sharding=sharding
        )
        for a in out_avals
    ]
    fn = bass2jax.fast_dispatch_compile(
        lambda: jax.jit(
            shard_map(
                _body,
                mesh=mesh,
                in_specs=(PartitionSpec("core"),) * (n_params + n_outs),
                out_specs=(PartitionSpec("core"),) * len(out_names),
                check_rep=False,
            ),
            donate_argnums=tuple(range(n_params, n_params + n_outs)),
            keep_unused=True,
        )
        .lower(*global_avals)
        .compile()
    )

    def stage(in_maps):
        concat_in = [
            np.concatenate(
                [np.asarray(in_maps[c][nm]) for c in range(NCORES)], axis=0
            )
            for nm in in_names
        ]
        zeros = [
            np.zeros((NCORES * z.shape[0], *z.shape[1:]), z.dtype)
            for z in zero_outs
        ]
        return concat_in, zeros

    def run(in_maps):
        import jax as _jax

        concat_in, zeros = stage(in_maps)
        concat_in = [_jax.device_put(a, sharding) for a in concat_in]
        zeros = [_jax.device_put(z, sharding) for z in zeros]
        out_arrs = fn(*concat_in, *zeros)
        out_arrs = [np.asarray(a) for a in _jax.block_until_ready(out_arrs)]
        return [
            {
                nm: out_arrs[i].reshape(NCORES, *out_avals[i].shape)[c]
                for i, nm in enumerate(out_names)
            }
            for c in range(NCORES)
        ]

    run.fn = fn
    run.stage = stage
    run.mesh = mesh
    run.sharding = sharding
    run.out_avals = out_avals
    run.out_names = out_names
    return run


def kernel(U_l, W):
    if "run" not in _CACHE:
        nc = build_program_v3(eng_b="dpdpdpdp", eng_c="dddddddd")
        _CACHE["nc"] = nc
        _CACHE["run"] = _build_executable(nc)
    in_maps = host_prep(U_l, W)
    results = _CACHE["run"](in_maps)
    out = np.concatenate(
        [results[c]["out_sh"].reshape(B // NCORES, NH, DH) for c in range(NCORES)],
        axis=0,
    )
    return out



# revision 37
# speedup vs baseline: 1.4428x; 1.0040x over previous
import sys

sys.path.insert(0, "/opt/trn_rl_repo")

import numpy as np

import concourse.bass as bass
import concourse.mybir as mybir
import concourse.tile as tile
from concourse import bacc

F32 = mybir.dt.float32
F32R = mybir.dt.float32r
BF16 = mybir.dt.bfloat16
AX = mybir.AxisListType
OP = mybir.AluOpType
AF = mybir.ActivationFunctionType

B, NL, NH, DL, DH = 64, 512, 32, 16, 32
NCORES = 8
ILOC = NL // NCORES
NG = ILOC // 8
NCB = B // 16
KL = NH * DH
ATT = 5.656854249492381
EPS = 1e-20
KREP = 32

_CACHE = {}


def _bcast(ap, n):
    return bass.AP(ap.tensor, ap.offset, list(ap.ap) + [[0, n]])


def _bcast_outer(ap, n):
    return bass.AP(
        ap.tensor, ap.offset,
        [list(ap.ap[0]), [0, n]] + [list(x) for x in ap.ap[1:]],
    )


def build_program_v3(
    krep=KREP,
    eng_b="dpdpdpdp",
    eng_c="dddddddd",
    uh_dt="f32",
    tmp_dt="f32",
    srep_dt="f32",
    cc_dt="bf16",
    coll="p",
    skip_ar=False,
    skip_rs=False,
):
    DT = {"f32": F32, "bf16": BF16}
    nc = bacc.Bacc(
        "TRN2",
        target_bir_lowering=False,
        debug=False,
        enable_asserts=False,
        num_devices=NCORES,
    )

    WUL = nc.dram_tensor("WUL", [NG, 128, 1600], F32, kind="ExternalInput")
    Ones = nc.dram_tensor("Ones", [NCB, 128, B], BF16, kind="ExternalInput")
    Bcast = nc.dram_tensor("Bcast", [NCB, B, 128], F32, kind="ExternalInput")
    out_sh = nc.dram_tensor("out_sh", [B // NCORES, KL], F32, kind="ExternalOutput")

    S_part = [nc.dram_tensor(f"S_part{i}", [B, KL], F32) for i in range(2)]
    S_full = [
        nc.dram_tensor(f"S_full{i}", [B, KL], F32, addr_space="Shared")
        for i in range(2)
    ]
    Uh_part = [nc.dram_tensor(f"Uh_part{i}", [B, KL], F32) for i in range(2)]
    Uh_my = [
        nc.dram_tensor(f"Uh_my{i}", [B // NCORES, KL], F32) for i in range(2)
    ]

    rg = [list(range(NCORES))]

    from contextlib import ExitStack

    with tile.TileContext(nc) as tc, ExitStack() as ctx:
        ENG = {"d": nc.vector, "p": nc.gpsimd}
        persist = ctx.enter_context(tc.tile_pool(name="persist", bufs=1))
        wulr_r = persist.tile([128, NG, 1600], F32R)
        srep = persist.tile([128, NCB, KL], DT[srep_dt])
        asum = persist.tile([128, NCB, NG, 32], F32)
        cc = persist.tile([128, NCB, NG, 32], DT[cc_dt])
        ones_sb = persist.tile([128, NCB, B], BF16)
        bc_sb = persist.tile([B, NCB, 128], F32)
        one_t = persist.tile([128, 1], F32)
        ps_uh_pool = ctx.enter_context(
            tc.tile_pool(name="psuh", bufs=1, space="PSUM")
        )

        with tc.tile_pool(name="wload", bufs=2) as wload:
            for g in range(NG):
                sc = wload.tile([128, 1600], F32, tag="wld")
                nc.sync.dma_start(out=sc[:], in_=WUL[g])
                nc.scalar.copy(out=wulr_r[:, g, :], in_=sc[:])
        nc.sync.dma_start(
            out=ones_sb[:],
            in_=Ones[:, :, :].rearrange("c p b -> p c b"),
        )
        nc.sync.dma_start(
            out=bc_sb[:], in_=Bcast[:, :, :].rearrange("c b m -> b c m")
        )
        nc.vector.memset(one_t[:], 1.0)

        ps_uh = ps_uh_pool.tile([B, KL], F32)
        nc.tensor.matmul(
            ps_uh[0:1, 0:1],
            lhsT=ones_sb[:, 0, 0:1],
            rhs=ones_sb[:, 0, 0:1],
            start=True,
            stop=True,
        )

        pools = {
            "small": ctx.enter_context(tc.tile_pool(name="small", bufs=1)),
            "psu": ctx.enter_context(
                tc.tile_pool(name="psu", bufs=3, space="PSUM")
            ),
            "uh": ctx.enter_context(tc.tile_pool(name="uh", bufs=2)),
            "tmp": ctx.enter_context(tc.tile_pool(name="tmp", bufs=2)),
        }

        env = dict(
            nc=nc, pools=pools, wulr=wulr_r, srep=srep, asum=asum, cc=cc,
            ones_sb=ones_sb, bc_sb=bc_sb, one_t=one_t, ps_uh=ps_uh,
            out_sh=out_sh, S_part=S_part, S_full=S_full, Uh_part=Uh_part,
            Uh_my=Uh_my, rg=rg, ENG=ENG, eng_b=eng_b, eng_c=eng_c,
            UH_DT=DT[uh_dt], TMP_DT=DT[tmp_dt], coll=coll,
            skip_ar=skip_ar, skip_rs=skip_rs, krep=krep, uh_tiles={},
        )
        _phase_s(env, 0)
        for rep in range(krep):
            _body_v3(env, rep)

    nc.finalize()
    return nc


def _phase_s(env, rep):
    nc = env["nc"]
    wulr = env["wulr"]
    small = env["pools"]["small"]
    psu_pool = env["pools"]["psu"]
    pb = rep % 2
    S_part, S_full = env["S_part"][pb], env["S_full"][pb]

    ps_s = psu_pool.tile([128, KL], F32, tag="psu")
    for g in range(NG):
        wr_g = wulr[:, g, 0:KL]
        ult_g = wulr[:, g, KL : KL + B]
        for nch in range(2):
            nc.tensor.matmul(
                ps_s[0:B, nch * 512 : (nch + 1) * 512],
                lhsT=ult_g,
                rhs=wr_g[:, nch * 512 : (nch + 1) * 512],
                start=(g == 0),
                stop=(g == NG - 1),
            )
    s_sb = small.tile([B, KL], F32, tag="s_sb")
    nc.scalar.copy(out=s_sb[:], in_=ps_s[0:B, :])
    nc.sync.dma_start(out=S_part[:, :], in_=s_sb[:])
    if env["skip_ar"]:
        nc.sync.dma_start(out=S_full[:, :], in_=S_part[:, :])
    else:
        nc.gpsimd.collective_compute(
            "AllReduce", OP.add, replica_groups=env["rg"],
            ins=[S_part[:, :]], outs=[S_full[:, :]],
        )


def _emit_uhat(env, key):
    nc = env["nc"]
    wulr = env["wulr"]
    psu_pool = env["pools"]["psu"]
    uh_pool = env["pools"]["uh"]
    rep, cb = key
    uh_cb = uh_pool.tile([128, NG, KL], env["UH_DT"], tag="uhcb")
    for g in range(NG):
        wr_g = wulr[:, g, 0:KL]
        lb_g = wulr[:, g, KL + B + 128 * cb : KL + B + 128 * (cb + 1)]
        psu = psu_pool.tile([128, KL], F32, tag="psu")
        for nch in range(2):
            nc.tensor.matmul(
                psu[:, nch * 512 : (nch + 1) * 512],
                lhsT=lb_g,
                rhs=wr_g[:, nch * 512 : (nch + 1) * 512],
                start=True,
                stop=True,
            )
        nc.scalar.copy(out=uh_cb[:, g, :], in_=psu[:])
    env["uh_tiles"][key] = uh_cb


def _body_v3(env, rep):
    nc = env["nc"]
    srep, asum, cc = env["srep"], env["asum"], env["cc"]
    ones_sb, bc_sb = env["ones_sb"], env["bc_sb"]
    ps_uh = env["ps_uh"]
    ENG, eng_b, eng_c = env["ENG"], env["eng_b"], env["eng_c"]
    TMP_DT = env["TMP_DT"]
    small = env["pools"]["small"]
    psu_pool = env["pools"]["psu"]
    tmp_pool = env["pools"]["tmp"]
    pb = rep % 2
    S_full = env["S_full"][pb]
    half = NG // 2

    _emit_uhat(env, (rep, 0))
    _emit_uhat(env, (rep, 1))

    sf_sb = small.tile([B, KL], F32, tag="sf_sb")
    nc.sync.dma_start(out=sf_sb[:], in_=S_full[:, :])
    for rcb in range(NCB):
        ps_r = psu_pool.tile([128, KL], F32, tag="psu")
        for nch in range(2):
            nc.tensor.matmul(
                ps_r[:, nch * 512 : (nch + 1) * 512],
                lhsT=bc_sb[:, rcb, :],
                rhs=sf_sb[:, nch * 512 : (nch + 1) * 512],
                start=True,
                stop=True,
            )
        nc.scalar.copy(out=srep[:, rcb, :], in_=ps_r[:])

    for cb in range(NCB):
        uh_cb = env["uh_tiles"].pop((rep, cb))
        for h in range(2):
            gs = slice(h * half, (h + 1) * half)
            tmp_h = tmp_pool.tile([128, half, 32, 32], TMP_DT, tag="tmpB")
            ENG[eng_b[cb * 2 + h]].tensor_tensor(
                tmp_h[:],
                uh_cb[:, gs].rearrange("p g (k l) -> p g k l", l=32),
                _bcast_outer(
                    srep[:, cb, :].rearrange("p (k l) -> p k l", l=32), half
                ),
                OP.mult,
            )
            nc.vector.tensor_reduce(
                asum[:, cb, gs], tmp_h[:], axis=AX.X, op=OP.add
            )
        mx = small.tile([128, NG], F32, tag="mx")
        nc.vector.tensor_reduce(mx[:], asum[:, cb], axis=AX.X, op=OP.max)
        zs = small.tile([128, NG, 32], F32, tag="zs")
        nc.vector.tensor_tensor(
            zs[:], asum[:, cb], _bcast(mx[:], 32), OP.subtract
        )
        ex = small.tile([128, NG, 32], F32, tag="ex")
        nc.scalar.activation(ex[:], zs[:], AF.Exp, scale=1.0 / ATT)
        sm = small.tile([128, NG], F32, tag="sm")
        nc.vector.tensor_reduce(sm[:], ex[:], axis=AX.X, op=OP.add)
        rc = small.tile([128, NG], F32, tag="rc")
        nc.vector.reciprocal(rc[:], sm[:])
        nc.vector.tensor_tensor(cc[:, cb], ex[:], _bcast(rc[:], 32), OP.mult)

        for h in range(2):
            gs = slice(h * half, (h + 1) * half)
            tmp2_h = tmp_pool.tile([128, half, 32, 32], BF16, tag="tmpC")
            ENG[eng_c[cb * 2 + h]].tensor_tensor(
                tmp2_h[:],
                uh_cb[:, gs].rearrange("p g (k l) -> p g k l", l=32),
                _bcast(cc[:, cb, gs], 32),
                OP.mult,
            )
            flat2 = tmp2_h[:].rearrange("p g a b -> p (g a b)")
            for nch in range(half * 2):
                nc.tensor.matmul(
                    ps_uh[:, (nch % 2) * 512 : (nch % 2 + 1) * 512],
                    lhsT=ones_sb[:, cb, :],
                    rhs=flat2[:, nch * 512 : (nch + 1) * 512],
                    start=(nch < 2 and h == 0 and cb == 0),
                    stop=(nch >= half * 2 - 2 and h == 1 and cb == NCB - 1),
                )
        if cb + 2 < NCB:
            _emit_uhat(env, (rep, cb + 2))

    _tail_v3(env, rep)


def _tail_v3(env, rep):
    nc = env["nc"]
    small = env["pools"]["small"]
    ps_uh = env["ps_uh"]
    one_t = env["one_t"]
    out_sh = env["out_sh"]
    rg = env["rg"]
    pb = rep % 2
    Uh_part, Uh_my = env["Uh_part"][pb], env["Uh_my"][pb]

    if rep + 1 < env["krep"]:
        _phase_s(env, rep + 1)

    uh_sb = small.tile([B, KL], F32, tag="stage4k")
    nc.scalar.copy(out=uh_sb[:], in_=ps_uh[:])
    nc.sync.dma_start(out=Uh_part[:, :], in_=uh_sb[:])

    if env["skip_rs"]:
        nc.sync.dma_start(out=Uh_my[:, :], in_=Uh_part[0 : B // NCORES, :])
    else:
        nc.gpsimd.collective_compute(
            "ReduceScatter", OP.add, replica_groups=rg,
            ins=[Uh_part[:, :]], outs=[Uh_my[:, :]],
        )
    for h in range(2):
        um = small.tile([128, DH], F32, tag=f"um{h}")
        nc.sync.dma_start(
            out=um[:],
            in_=Uh_my[:, :].rearrange("b (k l) -> (b k) l", l=DH)[
                128 * h : 128 * (h + 1), :
            ],
        )
        sq = small.tile([128, DH], F32, tag=f"sq{h}")
        nc.vector.tensor_tensor(sq[:], um[:], um[:], OP.mult)
        n2 = small.tile([128, 1], F32, tag=f"n2{h}")
        nc.vector.tensor_reduce(n2[:], sq[:], axis=AX.X, op=OP.add)
        nrm = small.tile([128, 1], F32, tag=f"nrm{h}")
        nc.scalar.activation(nrm[:], n2[:], AF.Sqrt)
        en = small.tile([128, 1], F32, tag=f"en{h}")
        nc.scalar.activation(en[:], nrm[:], AF.Exp, scale=-1.0)
        f1 = small.tile([128, 1], F32, tag=f"f1{h}")
        nc.vector.tensor_tensor(f1[:], one_t[:], en[:], OP.subtract)
        nd = small.tile([128, 1], F32, tag=f"nd{h}")
        nc.vector.tensor_scalar_add(nd[:], nrm[:], EPS)
        rn = small.tile([128, 1], F32, tag=f"rn{h}")
        nc.vector.reciprocal(rn[:], nd[:])
        fac = small.tile([128, 1], F32, tag=f"fac{h}")
        nc.vector.tensor_tensor(fac[:], f1[:], rn[:], OP.mult)
        ov = small.tile([128, DH], F32, tag=f"sq{h}")
        nc.vector.tensor_tensor(ov[:], um[:], _bcast(fac[:, 0], DH), OP.mult)
        nc.sync.dma_start(
            out=out_sh[:, :].rearrange("b (k l) -> (b k) l", l=DH)[
                128 * h : 128 * (h + 1), :
            ],
            in_=ov[:],
        )


def host_prep(U_l, W):
    U_l = np.asarray(U_l, dtype=np.float32)
    W = np.asarray(W, dtype=np.float32)
    import ml_dtypes

    ones = np.zeros((NCB, 128, B), dtype=ml_dtypes.bfloat16)
    for cb in range(NCB):
        for i_sub in range(8):
            ones[cb, 16 * i_sub : 16 * (i_sub + 1), 16 * cb : 16 * (cb + 1)] = np.eye(
                16, dtype=ml_dtypes.bfloat16
            )
    bcast = np.zeros((NCB, B, 128), dtype=np.float32)
    for cb in range(NCB):
        for i_sub in range(8):
            bcast[cb, 16 * cb : 16 * (cb + 1), 16 * i_sub : 16 * (i_sub + 1)] = np.eye(
                16, dtype=np.float32
            )
    in_maps = []
    for c in range(NCORES):
        i0 = c * ILOC
        Wsh = W[i0 : i0 + ILOC]
        Wr = np.ascontiguousarray(
            Wsh.reshape(NG, 8, NH, DL, DH).transpose(0, 1, 3, 2, 4)
        ).reshape(NG, 128, KL)
        Ush = U_l[:, i0 : i0 + ILOC, :]
        UlT = np.ascontiguousarray(
            Ush.reshape(B, NG, 8, DL).transpose(1, 2, 3, 0)
        ).reshape(NG, 128, B)
        Lb = np.zeros((NG, NCB, 128, 128), dtype=np.float32)
        blocks = UlT.reshape(NG, 8, DL, NCB, 16)
        for i_sub in range(8):
            Lb[:, :, 16 * i_sub : 16 * i_sub + DL, 16 * i_sub : 16 * (i_sub + 1)] = (
                blocks[:, i_sub].transpose(0, 2, 1, 3)
            )
        WUL = np.concatenate(
            [Wr, UlT, Lb.transpose(0, 2, 1, 3).reshape(NG, 128, NCB * 128)],
            axis=2,
        )
        in_maps.append({"WUL": WUL, "Ones": ones, "Bcast": bcast})
    return in_maps


def _build_executable(nc):
    import jax
    from jax.sharding import Mesh, PartitionSpec
    from jax.experimental.shard_map import shard_map
    from concourse import bass2jax

    bass2jax.install_neuronx_cc_hook()
    partition_name = nc.partition_id_tensor.name if nc.partition_id_tensor else None
    in_names, in_shapes, out_names, out_avals, zero_outs = [], [], [], [], []
    for alloc in nc.m.functions[0].allocations:
        if not isinstance(alloc, mybir.MemoryLocationSet):
            continue
        name = alloc.memorylocations[0].name
        if alloc.kind == "ExternalInput":
            if name != partition_name:
                in_names.append(name)
                in_shapes.append(
                    (tuple(alloc.tensor_shape), mybir.dt.np(alloc.dtype))
                )
        elif alloc.kind == "ExternalOutput":
            shape = tuple(alloc.tensor_shape)
            dtype = mybir.dt.np(alloc.dtype)
            out_names.append(name)
            out_avals.append(jax.core.ShapedArray(shape, dtype))
            zero_outs.append(np.zeros(shape, dtype))
    n_params = len(in_names)
    n_outs = len(out_avals)
    all_names = list(in_names) + out_names
    if partition_name is not None:
        all_names.append(partition_name)

    def _body(*args):
        operands = list(args)
        if partition_name is not None:
            operands.append(bass2jax.partition_id_tensor())
        outs = bass2jax._bass_exec_p.bind(
            *operands,
            out_avals=tuple(out_avals),
            in_names=tuple(all_names),
            out_names=tuple(out_names),
            lowering_input_output_aliases=(),
            sim_require_finite=True,
            sim_require_nnan=True,
            nc=nc,
        )
        return tuple(outs)

    devices = jax.devices()[:NCORES]
    mesh = Mesh(np.asarray(devices), ("core",))
    sharding = jax.sharding.NamedSharding(mesh, PartitionSpec("core"))
    global_avals = [
        jax.ShapeDtypeStruct((NCORES * s[0], *s[1:]), dt, sharding=sharding)
        for (s, dt) in in_shapes
    ] + [
        jax.ShapeDtypeStruct(
            (NCORES * a.shape[0], *a.shape[1:]), a.dtype, sharding=sharding
        )
        for a in out_avals
    ]
    fn = bass2jax.fast_dispatch_compile(
        lambda: jax.jit(
            shard_map(
                _body,
                mesh=mesh,
                in_specs=(PartitionSpec("core"),) * (n_params + n_outs),
                out_specs=(PartitionSpec("core"),) * len(out_names),
                check_rep=False,
            ),
            donate_argnums=tuple(range(n_params, n_params + n_outs)),
            keep_unused=True,
        )
        .lower(*global_avals)
        .compile()
    )

    def stage(in_maps):
        concat_in = [
            np.concatenate(
                [np.asarray(in_maps[c][nm]) for c in range(NCORES)], axis=0
            )
            for nm in in_names
        ]
        zeros = [
            np.zeros((NCORES * z.shape[0], *z.shape[1:]), z.dtype)
            for z in zero_outs
        ]
        return concat_in, zeros

    def run(in_maps):
        import jax as _jax

        concat_in, zeros = stage(in_maps)
        concat_in = [_jax.device_put(a, sharding) for a in concat_in]
        zeros = [_jax.device_put(z, sharding) for z in zeros]
        out_arrs = fn(*concat_in, *zeros)
        out_arrs = [np.asarray(a) for a in _jax.block_until_ready(out_arrs)]
        return [
            {
                nm: out_arrs[i].reshape(NCORES, *out_avals[i].shape)[c]
                for i, nm in enumerate(out_names)
            }
            for c in range(NCORES)
        ]

    run.fn = fn
    run.stage = stage
    run.mesh = mesh
    run.sharding = sharding
    run.out_avals = out_avals
    run.out_names = out_names
    return run


def kernel(U_l, W):
    if "run" not in _CACHE:
        nc = build_program_v3(eng_b="dpdpdpdp", eng_c="dddddddd")
        _CACHE["nc"] = nc
        _CACHE["run"] = _build_executable(nc)
    in_maps = host_prep(U_l, W)
    results = _CACHE["run"](in_maps)
    out = np.concatenate(
        [results[c]["out_sh"].reshape(B // NCORES, NH, DH) for c in range(NCORES)],
        axis=0,
    )
    return out


# revision 39
# speedup vs baseline: 1.4622x; 1.0134x over previous
import sys

sys.path.insert(0, "/opt/trn_rl_repo")

import numpy as np

import concourse.bass as bass
import concourse.mybir as mybir
import concourse.tile as tile
from concourse import bacc

F32 = mybir.dt.float32
F32R = mybir.dt.float32r
BF16 = mybir.dt.bfloat16
AX = mybir.AxisListType
OP = mybir.AluOpType
AF = mybir.ActivationFunctionType

B, NL, NH, DL, DH = 64, 512, 32, 16, 32
NCORES = 8
ILOC = NL // NCORES
NG = ILOC // 8
NCB = B // 16
KL = NH * DH
ATT = 5.656854249492381
EPS = 1e-20
KREP = 32

_CACHE = {}


def _bcast(ap, n):
    return bass.AP(ap.tensor, ap.offset, list(ap.ap) + [[0, n]])


def _bcast_outer(ap, n):
    return bass.AP(
        ap.tensor, ap.offset,
        [list(ap.ap[0]), [0, n]] + [list(x) for x in ap.ap[1:]],
    )


def build_program_v3(
    krep=KREP,
    eng_b="dddddddd",
    eng_c="dddddddd",
    uh_dt="f32",
    tmp_dt="f32",
    srep_dt="f32",
    cc_dt="bf16",
    coll="p",
    skip_ar=False,
    skip_rs=False,
):
    DT = {"f32": F32, "bf16": BF16}
    nc = bacc.Bacc(
        "TRN2",
        target_bir_lowering=False,
        debug=False,
        enable_asserts=False,
        num_devices=NCORES,
    )

    WUL = nc.dram_tensor("WUL", [NG, 128, 1600], F32, kind="ExternalInput")
    Ones = nc.dram_tensor("Ones", [NCB, 128, B], BF16, kind="ExternalInput")
    Bcast = nc.dram_tensor("Bcast", [NCB, B, 128], F32, kind="ExternalInput")
    out_sh = nc.dram_tensor("out_sh", [B // NCORES, KL], F32, kind="ExternalOutput")

    S_part = [nc.dram_tensor(f"S_part{i}", [B, KL], F32) for i in range(2)]
    S_full = [
        nc.dram_tensor(f"S_full{i}", [B, KL], F32, addr_space="Shared")
        for i in range(2)
    ]
    Uh_part = [nc.dram_tensor(f"Uh_part{i}", [B, KL], F32) for i in range(2)]
    Uh_my = [
        nc.dram_tensor(f"Uh_my{i}", [B // NCORES, KL], F32) for i in range(2)
    ]

    rg = [list(range(NCORES))]

    from contextlib import ExitStack

    with tile.TileContext(nc) as tc, ExitStack() as ctx:
        ENG = {"d": nc.vector, "p": nc.gpsimd}
        persist = ctx.enter_context(tc.tile_pool(name="persist", bufs=1))
        wulr_r = persist.tile([128, NG, 1600], F32R)
        srep = persist.tile([128, NCB, KL], DT[srep_dt])
        asum = persist.tile([128, NCB, NG, 32], F32)
        cc = persist.tile([128, NCB, NG, 32], DT[cc_dt])
        ones_sb = persist.tile([128, NCB, B], BF16)
        bc_sb = persist.tile([B, NCB, 128], F32)
        one_t = persist.tile([128, 1], F32)
        ps_uh_pool = ctx.enter_context(
            tc.tile_pool(name="psuh", bufs=1, space="PSUM")
        )

        with tc.tile_pool(name="wload", bufs=2) as wload:
            for g in range(NG):
                sc = wload.tile([128, 1600], F32, tag="wld")
                nc.sync.dma_start(out=sc[:], in_=WUL[g])
                nc.scalar.copy(out=wulr_r[:, g, :], in_=sc[:])
        nc.sync.dma_start(
            out=ones_sb[:],
            in_=Ones[:, :, :].rearrange("c p b -> p c b"),
        )
        nc.sync.dma_start(
            out=bc_sb[:], in_=Bcast[:, :, :].rearrange("c b m -> b c m")
        )
        nc.vector.memset(one_t[:], 1.0)

        ps_uh = ps_uh_pool.tile([B, KL], F32)
        nc.tensor.matmul(
            ps_uh[0:1, 0:1],
            lhsT=ones_sb[:, 0, 0:1],
            rhs=ones_sb[:, 0, 0:1],
            start=True,
            stop=True,
        )

        pools = {
            "small": ctx.enter_context(tc.tile_pool(name="small", bufs=1)),
            "psu": ctx.enter_context(
                tc.tile_pool(name="psu", bufs=3, space="PSUM")
            ),
            "uh": ctx.enter_context(tc.tile_pool(name="uh", bufs=2)),
            "tmp": ctx.enter_context(tc.tile_pool(name="tmp", bufs=2)),
        }

        env = dict(
            nc=nc, pools=pools, wulr=wulr_r, srep=srep, asum=asum, cc=cc,
            ones_sb=ones_sb, bc_sb=bc_sb, one_t=one_t, ps_uh=ps_uh,
            out_sh=out_sh, S_part=S_part, S_full=S_full, Uh_part=Uh_part,
            Uh_my=Uh_my, rg=rg, ENG=ENG, eng_b=eng_b, eng_c=eng_c,
            UH_DT=DT[uh_dt], TMP_DT=DT[tmp_dt], coll=coll,
            skip_ar=skip_ar, skip_rs=skip_rs, krep=krep, uh_tiles={},
        )
        _phase_s(env, 0)
        for rep in range(krep):
            _body_v3(env, rep)

    nc.finalize()
    return nc


def _phase_s(env, rep):
    nc = env["nc"]
    wulr = env["wulr"]
    small = env["pools"]["small"]
    psu_pool = env["pools"]["psu"]
    pb = rep % 2
    S_part, S_full = env["S_part"][pb], env["S_full"][pb]

    ps_s = psu_pool.tile([128, KL], F32, tag="psu")
    for g in range(NG):
        wr_g = wulr[:, g, 0:KL]
        ult_g = wulr[:, g, KL : KL + B]
        for nch in range(2):
            nc.tensor.matmul(
                ps_s[0:B, nch * 512 : (nch + 1) * 512],
                lhsT=ult_g,
                rhs=wr_g[:, nch * 512 : (nch + 1) * 512],
                start=(g == 0),
                stop=(g == NG - 1),
            )
    s_sb = small.tile([B, KL], F32, tag="s_sb")
    nc.scalar.copy(out=s_sb[:], in_=ps_s[0:B, :])
    nc.sync.dma_start(out=S_part[:, :], in_=s_sb[:])
    if env["skip_ar"]:
        nc.sync.dma_start(out=S_full[:, :], in_=S_part[:, :])
    else:
        nc.gpsimd.collective_compute(
            "AllReduce", OP.add, replica_groups=env["rg"],
            ins=[S_part[:, :]], outs=[S_full[:, :]],
        )


def _emit_uhat(env, key):
    nc = env["nc"]
    wulr = env["wulr"]
    psu_pool = env["pools"]["psu"]
    uh_pool = env["pools"]["uh"]
    rep, cb = key
    uh_cb = uh_pool.tile([128, NG, KL], env["UH_DT"], tag="uhcb")
    for g in range(NG):
        wr_g = wulr[:, g, 0:KL]
        lb_g = wulr[:, g, KL + B + 128 * cb : KL + B + 128 * (cb + 1)]
        psu = psu_pool.tile([128, KL], F32, tag="psu")
        for nch in range(2):
            nc.tensor.matmul(
                psu[:, nch * 512 : (nch + 1) * 512],
                lhsT=lb_g,
                rhs=wr_g[:, nch * 512 : (nch + 1) * 512],
                start=True,
                stop=True,
            )
        nc.scalar.copy(out=uh_cb[:, g, :], in_=psu[:])
    env["uh_tiles"][key] = uh_cb


def _body_v3(env, rep):
    nc = env["nc"]
    srep, asum, cc = env["srep"], env["asum"], env["cc"]
    ones_sb, bc_sb = env["ones_sb"], env["bc_sb"]
    ps_uh = env["ps_uh"]
    ENG, eng_b, eng_c = env["ENG"], env["eng_b"], env["eng_c"]
    TMP_DT = env["TMP_DT"]
    small = env["pools"]["small"]
    psu_pool = env["pools"]["psu"]
    tmp_pool = env["pools"]["tmp"]
    pb = rep % 2
    S_full = env["S_full"][pb]
    half = NG // 2

    _emit_uhat(env, (rep, 0))
    _emit_uhat(env, (rep, 1))

    sf_sb = small.tile([B, KL], F32, tag="sf_sb")
    nc.sync.dma_start(out=sf_sb[:], in_=S_full[:, :])
    for rcb in range(NCB):
        ps_r = psu_pool.tile([128, KL], F32, tag="psu")
        for nch in range(2):
            nc.tensor.matmul(
                ps_r[:, nch * 512 : (nch + 1) * 512],
                lhsT=bc_sb[:, rcb, :],
                rhs=sf_sb[:, nch * 512 : (nch + 1) * 512],
                start=True,
                stop=True,
            )
        nc.scalar.copy(out=srep[:, rcb, :], in_=ps_r[:])

    def bfront(cb):
        uh_cb = env["uh_tiles"][(rep, cb)]
        for h in range(2):
            gs = slice(h * half, (h + 1) * half)
            tmp_h = tmp_pool.tile([128, half, 32, 32], TMP_DT, tag="tmpB")
            ENG[eng_b[cb * 2 + h]].tensor_tensor(
                tmp_h[:],
                uh_cb[:, gs].rearrange("p g (k l) -> p g k l", l=32),
                _bcast_outer(
                    srep[:, cb, :].rearrange("p (k l) -> p k l", l=32), half
                ),
                OP.mult,
            )
            nc.vector.tensor_reduce(
                asum[:, cb, gs], tmp_h[:], axis=AX.X, op=OP.add
            )
        mx = small.tile([128, NG], F32, tag="mx")
        nc.vector.tensor_reduce(mx[:], asum[:, cb], axis=AX.X, op=OP.max)
        zs = small.tile([128, NG, 32], F32, tag="zs")
        nc.vector.tensor_tensor(
            zs[:], asum[:, cb], _bcast(mx[:], 32), OP.subtract
        )
        ex = small.tile([128, NG, 32], F32, tag=f"ex{cb % 2}")
        nc.scalar.activation(ex[:], zs[:], AF.Exp, scale=1.0 / ATT)
        return ex

    def ctail(cb, ex):
        uh_cb = env["uh_tiles"].pop((rep, cb))
        sm = small.tile([128, NG], F32, tag="sm")
        nc.vector.tensor_reduce(sm[:], ex[:], axis=AX.X, op=OP.add)
        rc = small.tile([128, NG], F32, tag="rc")
        nc.vector.reciprocal(rc[:], sm[:])
        nc.vector.tensor_tensor(cc[:, cb], ex[:], _bcast(rc[:], 32), OP.mult)
        for h in range(2):
            gs = slice(h * half, (h + 1) * half)
            tmp2_h = tmp_pool.tile([128, half, 32, 32], BF16, tag="tmpC")
            ENG[eng_c[cb * 2 + h]].tensor_tensor(
                tmp2_h[:],
                uh_cb[:, gs].rearrange("p g (k l) -> p g k l", l=32),
                _bcast(cc[:, cb, gs], 32),
                OP.mult,
            )
            flat2 = tmp2_h[:].rearrange("p g a b -> p (g a b)")
            for nch in range(half * 2):
                nc.tensor.matmul(
                    ps_uh[:, (nch % 2) * 512 : (nch % 2 + 1) * 512],
                    lhsT=ones_sb[:, cb, :],
                    rhs=flat2[:, nch * 512 : (nch + 1) * 512],
                    start=(nch < 2 and h == 0 and cb == 0),
                    stop=(nch >= half * 2 - 2 and h == 1 and cb == NCB - 1),
                )
        if cb + 2 < NCB:
            _emit_uhat(env, (rep, cb + 2))

    ex_prev = bfront(0)
    for cb in range(NCB):
        ex_next = bfront(cb + 1) if cb + 1 < NCB else None
        ctail(cb, ex_prev)
        ex_prev = ex_next

    _tail_v3(env, rep)


def _tail_v3(env, rep):
    nc = env["nc"]
    small = env["pools"]["small"]
    ps_uh = env["ps_uh"]
    one_t = env["one_t"]
    out_sh = env["out_sh"]
    rg = env["rg"]
    pb = rep % 2
    Uh_part, Uh_my = env["Uh_part"][pb], env["Uh_my"][pb]

    if rep + 1 < env["krep"]:
        _phase_s(env, rep + 1)

    uh_sb = small.tile([B, KL], F32, tag="stage4k")
    nc.scalar.copy(out=uh_sb[:], in_=ps_uh[:])
    nc.sync.dma_start(out=Uh_part[:, :], in_=uh_sb[:])

    if env["skip_rs"]:
        nc.sync.dma_start(out=Uh_my[:, :], in_=Uh_part[0 : B // NCORES, :])
    else:
        nc.gpsimd.collective_compute(
            "ReduceScatter", OP.add, replica_groups=rg,
            ins=[Uh_part[:, :]], outs=[Uh_my[:, :]],
        )
    for h in range(2):
        um = small.tile([128, DH], F32, tag=f"um{h}")
        nc.sync.dma_start(
            out=um[:],
            in_=Uh_my[:, :].rearrange("b (k l) -> (b k) l", l=DH)[
                128 * h : 128 * (h + 1), :
            ],
        )
        sq = small.tile([128, DH], F32, tag=f"sq{h}")
        nc.vector.tensor_tensor(sq[:], um[:], um[:], OP.mult)
        n2 = small.tile([128, 1], F32, tag=f"n2{h}")
        nc.vector.tensor_reduce(n2[:], sq[:], axis=AX.X, op=OP.add)
        nrm = small.tile([128, 1], F32, tag=f"nrm{h}")
        nc.scalar.activation(nrm[:], n2[:], AF.Sqrt)
        en = small.tile([128, 1], F32, tag=f"en{h}")
        nc.scalar.activation(en[:], nrm[:], AF.Exp, scale=-1.0)
        f1 = small.tile([128, 1], F32, tag=f"f1{h}")
        nc.vector.tensor_tensor(f1[:], one_t[:], en[:], OP.subtract)
        nd = small.tile([128, 1], F32, tag=f"nd{h}")
        nc.vector.tensor_scalar_add(nd[:], nrm[:], EPS)
        rn = small.tile([128, 1], F32, tag=f"rn{h}")
        nc.vector.reciprocal(rn[:], nd[:])
        fac = small.tile([128, 1], F32, tag=f"fac{h}")
        nc.vector.tensor_tensor(fac[:], f1[:], rn[:], OP.mult)
        ov = small.tile([128, DH], F32, tag=f"sq{h}")
        nc.vector.tensor_tensor(ov[:], um[:], _bcast(fac[:, 0], DH), OP.mult)
        nc.sync.dma_start(
            out=out_sh[:, :].rearrange("b (k l) -> (b k) l", l=DH)[
                128 * h : 128 * (h + 1), :
            ],
            in_=ov[:],
        )


def host_prep(U_l, W):
    U_l = np.asarray(U_l, dtype=np.float32)
    W = np.asarray(W, dtype=np.float32)
    import ml_dtypes

    ones = np.zeros((NCB, 128, B), dtype=ml_dtypes.bfloat16)
    for cb in range(NCB):
        for i_sub in range(8):
            ones[cb, 16 * i_sub : 16 * (i_sub + 1), 16 * cb : 16 * (cb + 1)] = np.eye(
                16, dtype=ml_dtypes.bfloat16
            )
    bcast = np.zeros((NCB, B, 128), dtype=np.float32)
    for cb in range(NCB):
        for i_sub in range(8):
            bcast[cb, 16 * cb : 16 * (cb + 1), 16 * i_sub : 16 * (i_sub + 1)] = np.eye(
                16, dtype=np.float32
            )
    in_maps = []
    for c in range(NCORES):
        i0 = c * ILOC
        Wsh = W[i0 : i0 + ILOC]
        Wr = np.ascontiguousarray(
            Wsh.reshape(NG, 8, NH, DL, DH).transpose(0, 1, 3, 2, 4)
        ).reshape(NG, 128, KL)
        Ush = U_l[:, i0 : i0 + ILOC, :]
        UlT = np.ascontiguousarray(
            Ush.reshape(B, NG, 8, DL).transpose(1, 2, 3, 0)
        ).reshape(NG, 128, B)
        Lb = np.zeros((NG, NCB, 128, 128), dtype=np.float32)
        blocks = UlT.reshape(NG, 8, DL, NCB, 16)
        for i_sub in range(8):
            Lb[:, :, 16 * i_sub : 16 * i_sub + DL, 16 * i_sub : 16 * (i_sub + 1)] = (
                blocks[:, i_sub].transpose(0, 2, 1, 3)
            )
        WUL = np.concatenate(
            [Wr, UlT, Lb.transpose(0, 2, 1, 3).reshape(NG, 128, NCB * 128)],
            axis=2,
        )
        in_maps.append({"WUL": WUL, "Ones": ones, "Bcast": bcast})
    return in_maps


def _build_executable(nc):
    import jax
    from jax.sharding import Mesh, PartitionSpec
    from jax.experimental.shard_map import shard_map
    from concourse import bass2jax

    bass2jax.install_neuronx_cc_hook()
    partition_name = nc.partition_id_tensor.name if nc.partition_id_tensor else None
    in_names, in_shapes, out_names, out_avals, zero_outs = [], [], [], [], []
    for alloc in nc.m.functions[0].allocations:
        if not isinstance(alloc, mybir.MemoryLocationSet):
            continue
        name = alloc.memorylocations[0].name
        if alloc.kind == "ExternalInput":
            if name != partition_name:
                in_names.append(name)
                in_shapes.append(
                    (tuple(alloc.tensor_shape), mybir.dt.np(alloc.dtype))
                )
        elif alloc.kind == "ExternalOutput":
            shape = tuple(alloc.tensor_shape)
            dtype = mybir.dt.np(alloc.dtype)
            out_names.append(name)
            out_avals.append(jax.core.ShapedArray(shape, dtype))
            zero_outs.append(np.zeros(shape, dtype))
    n_params = len(in_names)
    n_outs = len(out_avals)
    all_names = list(in_names) + out_names
    if partition_name is not None:
        all_names.append(partition_name)

    def _body(*args):
        operands = list(args)
        if partition_name is not None:
            operands.append(bass2jax.partition_id_tensor())
        outs = bass2jax._bass_exec_p.bind(
            *operands,
            out_avals=tuple(out_avals),
            in_names=tuple(all_names),
            out_names=tuple(out_names),
            lowering_input_output_aliases=(),
            sim_require_finite=True,
            sim_require_nnan=True,
            nc=nc,
        )
        return tuple(outs)

    devices = jax.devices()[:NCORES]
    mesh = Mesh(np.asarray(devices), ("core",))
    sharding = jax.sharding.NamedSharding(mesh, PartitionSpec("core"))
    global_avals = [
        jax.ShapeDtypeStruct((NCORES * s[0], *s[1:]), dt, sharding=sharding)
        for (s, dt) in in_shapes
    ] + [
        jax.ShapeDtypeStruct(
            (NCORES * a.shape[0], *a.shape[1:]), a.dtype, sharding=sharding
        )
        for a in out_avals
    ]
    fn = bass2jax.fast_dispatch_compile(
        lambda: jax.jit(
            shard_map(
                _body,
                mesh=mesh,
                in_specs=(PartitionSpec("core"),) * (n_params + n_outs),
                out_specs=(PartitionSpec("core"),) * len(out_names),
                check_rep=False,
            ),
            donate_argnums=tuple(range(n_params, n_params + n_outs)),
            keep_unused=True,
        )
        .lower(*global_avals)
        .compile()
    )

    def stage(in_maps):
        concat_in = [
            np.concatenate(
                [np.asarray(in_maps[c][nm]) for c in range(NCORES)], axis=0
            )
            for nm in in_names
        ]
        zeros = [
            np.zeros((NCORES * z.shape[0], *z.shape[1:]), z.dtype)
            for z in zero_outs
        ]
        return concat_in, zeros

    def run(in_maps):
        import jax as _jax

        concat_in, zeros = stage(in_maps)
        concat_in = [_jax.device_put(a, sharding) for a in concat_in]
        zeros = [_jax.device_put(z, sharding) for z in zeros]
        out_arrs = fn(*concat_in, *zeros)
        out_arrs = [np.asarray(a) for a in _jax.block_until_ready(out_arrs)]
        return [
            {
                nm: out_arrs[i].reshape(NCORES, *out_avals[i].shape)[c]
                for i, nm in enumerate(out_names)
            }
            for c in range(NCORES)
        ]

    run.fn = fn
    run.stage = stage
    run.mesh = mesh
    run.sharding = sharding
    run.out_avals = out_avals
    run.out_names = out_names
    return run


def kernel(U_l, W):
    if "run" not in _CACHE:
        nc = build_program_v3(eng_b="dddddddd", eng_c="dddddddd")
        _CACHE["nc"] = nc
        _CACHE["run"] = _build_executable(nc)
    in_maps = host_prep(U_l, W)
    results = _CACHE["run"](in_maps)
    out = np.concatenate(
        [results[c]["out_sh"].reshape(B // NCORES, NH, DH) for c in range(NCORES)],
        axis=0,
    )
    return out


# revision 41
# speedup vs baseline: 1.5169x; 1.0374x over previous
import sys

sys.path.insert(0, "/opt/trn_rl_repo")

import numpy as np

import concourse.bass as bass
import concourse.mybir as mybir
import concourse.tile as tile
from concourse import bacc

F32 = mybir.dt.float32
F32R = mybir.dt.float32r
BF16 = mybir.dt.bfloat16
AX = mybir.AxisListType
OP = mybir.AluOpType
AF = mybir.ActivationFunctionType

B, NL, NH, DL, DH = 64, 512, 32, 16, 32
NCORES = 8
ILOC = NL // NCORES
NG = ILOC // 8
NCB = B // 16
KL = NH * DH
ATT = 5.656854249492381
EPS = 1e-20
KREP = 64

_CACHE = {}


def _bcast(ap, n):
    return bass.AP(ap.tensor, ap.offset, list(ap.ap) + [[0, n]])


def _bcast_outer(ap, n):
    return bass.AP(
        ap.tensor, ap.offset,
        [list(ap.ap[0]), [0, n]] + [list(x) for x in ap.ap[1:]],
    )


def build_program_v3(
    krep=KREP,
    eng_b="dddddddd",
    eng_c="dddddddd",
    uh_dt="f32",
    tmp_dt="f32",
    srep_dt="f32",
    cc_dt="bf16",
    coll="p",
    tmp_bufs=2,
    skip_ar=False,
    skip_rs=False,
):
    DT = {"f32": F32, "bf16": BF16}
    nc = bacc.Bacc(
        "TRN2",
        target_bir_lowering=False,
        debug=False,
        enable_asserts=False,
        num_devices=NCORES,
    )

    WUL = nc.dram_tensor("WUL", [NG, 128, 1600], F32, kind="ExternalInput")
    Ones = nc.dram_tensor("Ones", [NCB, 128, B], BF16, kind="ExternalInput")
    Bcast = nc.dram_tensor("Bcast", [NCB, B, 128], F32, kind="ExternalInput")
    out_sh = nc.dram_tensor("out_sh", [B // NCORES, KL], F32, kind="ExternalOutput")

    S_part = [nc.dram_tensor(f"S_part{i}", [B, KL], F32) for i in range(2)]
    S_full = [
        nc.dram_tensor(f"S_full{i}", [B, KL], F32, addr_space="Shared")
        for i in range(2)
    ]
    Uh_part = [nc.dram_tensor(f"Uh_part{i}", [B, KL], F32) for i in range(2)]
    Uh_my = [
        nc.dram_tensor(f"Uh_my{i}", [B // NCORES, KL], F32) for i in range(2)
    ]

    rg = [list(range(NCORES))]

    from contextlib import ExitStack

    with tile.TileContext(nc) as tc, ExitStack() as ctx:
        ENG = {"d": nc.vector, "p": nc.gpsimd}
        persist = ctx.enter_context(tc.tile_pool(name="persist", bufs=1))
        wulr_r = persist.tile([128, NG, 1600], F32R)
        srep = persist.tile([128, NCB, KL], DT[srep_dt])
        asum = persist.tile([128, NCB, NG, 32], F32)
        cc = persist.tile([128, NCB, NG, 32], DT[cc_dt])
        ones_sb = persist.tile([128, NCB, B], BF16)
        bc_sb = persist.tile([B, NCB, 128], F32)
        one_t = persist.tile([128, 1], F32)
        ps_uh_pool = ctx.enter_context(
            tc.tile_pool(name="psuh", bufs=1, space="PSUM")
        )

        with tc.tile_pool(name="wload", bufs=2) as wload:
            for g in range(NG):
                sc = wload.tile([128, 1600], F32, tag="wld")
                nc.sync.dma_start(out=sc[:], in_=WUL[g])
                nc.scalar.copy(out=wulr_r[:, g, :], in_=sc[:])
        nc.sync.dma_start(
            out=ones_sb[:],
            in_=Ones[:, :, :].rearrange("c p b -> p c b"),
        )
        nc.sync.dma_start(
            out=bc_sb[:], in_=Bcast[:, :, :].rearrange("c b m -> b c m")
        )
        nc.vector.memset(one_t[:], 1.0)

        ps_uh = ps_uh_pool.tile([B, KL], F32)
        nc.tensor.matmul(
            ps_uh[0:1, 0:1],
            lhsT=ones_sb[:, 0, 0:1],
            rhs=ones_sb[:, 0, 0:1],
            start=True,
            stop=True,
        )

        pools = {
            "small": ctx.enter_context(tc.tile_pool(name="small", bufs=1)),
            "psu": ctx.enter_context(
                tc.tile_pool(name="psu", bufs=3, space="PSUM")
            ),
            "uh": ctx.enter_context(tc.tile_pool(name="uh", bufs=2)),
            "tmp": ctx.enter_context(tc.tile_pool(name="tmp", bufs=tmp_bufs)),
        }

        env = dict(
            nc=nc, pools=pools, wulr=wulr_r, srep=srep, asum=asum, cc=cc,
            ones_sb=ones_sb, bc_sb=bc_sb, one_t=one_t, ps_uh=ps_uh,
            out_sh=out_sh, S_part=S_part, S_full=S_full, Uh_part=Uh_part,
            Uh_my=Uh_my, rg=rg, ENG=ENG, eng_b=eng_b, eng_c=eng_c,
            UH_DT=DT[uh_dt], TMP_DT=DT[tmp_dt], coll=coll,
            skip_ar=skip_ar, skip_rs=skip_rs, krep=krep, uh_tiles={},
        )
        _phase_s(env, 0)
        for rep in range(krep):
            _body_v3(env, rep)

    nc.finalize()
    return nc


def _phase_s(env, rep):
    nc = env["nc"]
    wulr = env["wulr"]
    small = env["pools"]["small"]
    psu_pool = env["pools"]["psu"]
    pb = rep % 2
    S_part, S_full = env["S_part"][pb], env["S_full"][pb]

    ps_s = psu_pool.tile([128, KL], F32, tag="psu")
    for g in range(NG):
        wr_g = wulr[:, g, 0:KL]
        ult_g = wulr[:, g, KL : KL + B]
        for nch in range(2):
            nc.tensor.matmul(
                ps_s[0:B, nch * 512 : (nch + 1) * 512],
                lhsT=ult_g,
                rhs=wr_g[:, nch * 512 : (nch + 1) * 512],
                start=(g == 0),
                stop=(g == NG - 1),
            )
    s_sb = small.tile([B, KL], F32, tag="s_sb")
    nc.scalar.copy(out=s_sb[:], in_=ps_s[0:B, :])
    nc.sync.dma_start(out=S_part[:, :], in_=s_sb[:])
    if env["skip_ar"]:
        nc.sync.dma_start(out=S_full[:, :], in_=S_part[:, :])
    else:
        nc.gpsimd.collective_compute(
            "AllReduce", OP.add, replica_groups=env["rg"],
            ins=[S_part[:, :]], outs=[S_full[:, :]],
        )


def _emit_uhat(env, key):
    nc = env["nc"]
    wulr = env["wulr"]
    psu_pool = env["pools"]["psu"]
    uh_pool = env["pools"]["uh"]
    rep, cb = key
    uh_cb = uh_pool.tile([128, NG, KL], env["UH_DT"], tag="uhcb")
    for g in range(NG):
        wr_g = wulr[:, g, 0:KL]
        lb_g = wulr[:, g, KL + B + 128 * cb : KL + B + 128 * (cb + 1)]
        psu = psu_pool.tile([128, KL], F32, tag="psu")
        for nch in range(2):
            nc.tensor.matmul(
                psu[:, nch * 512 : (nch + 1) * 512],
                lhsT=lb_g,
                rhs=wr_g[:, nch * 512 : (nch + 1) * 512],
                start=True,
                stop=True,
            )
        nc.scalar.copy(out=uh_cb[:, g, :], in_=psu[:])
    env["uh_tiles"][key] = uh_cb


def _body_v3(env, rep):
    nc = env["nc"]
    srep, asum, cc = env["srep"], env["asum"], env["cc"]
    ones_sb, bc_sb = env["ones_sb"], env["bc_sb"]
    ps_uh = env["ps_uh"]
    ENG, eng_b, eng_c = env["ENG"], env["eng_b"], env["eng_c"]
    TMP_DT = env["TMP_DT"]
    small = env["pools"]["small"]
    psu_pool = env["pools"]["psu"]
    tmp_pool = env["pools"]["tmp"]
    pb = rep % 2
    S_full = env["S_full"][pb]
    half = NG // 2

    _emit_uhat(env, (rep, 0))
    _emit_uhat(env, (rep, 1))

    sf_sb = small.tile([B, KL], F32, tag="sf_sb")
    nc.sync.dma_start(out=sf_sb[:], in_=S_full[:, :])
    for rcb in range(NCB):
        ps_r = psu_pool.tile([128, KL], F32, tag="psu")
        for nch in range(2):
            nc.tensor.matmul(
                ps_r[:, nch * 512 : (nch + 1) * 512],
                lhsT=bc_sb[:, rcb, :],
                rhs=sf_sb[:, nch * 512 : (nch + 1) * 512],
                start=True,
                stop=True,
            )
        nc.scalar.copy(out=srep[:, rcb, :], in_=ps_r[:])

    def bfront(cb):
        uh_cb = env["uh_tiles"][(rep, cb)]
        for h in range(2):
            gs = slice(h * half, (h + 1) * half)
            tmp_h = tmp_pool.tile([128, half, 32, 32], TMP_DT, tag="tmpB")
            ENG[eng_b[cb * 2 + h]].tensor_tensor(
                tmp_h[:],
                uh_cb[:, gs].rearrange("p g (k l) -> p g k l", l=32),
                _bcast_outer(
                    srep[:, cb, :].rearrange("p (k l) -> p k l", l=32), half
                ),
                OP.mult,
            )
            nc.vector.tensor_reduce(
                asum[:, cb, gs], tmp_h[:], axis=AX.X, op=OP.add
            )
        mx = small.tile([128, NG], F32, tag="mx")
        nc.vector.tensor_reduce(mx[:], asum[:, cb], axis=AX.X, op=OP.max)
        zs = small.tile([128, NG, 32], F32, tag="zs")
        nc.vector.tensor_tensor(
            zs[:], asum[:, cb], _bcast(mx[:], 32), OP.subtract
        )
        ex = small.tile([128, NG, 32], F32, tag=f"ex{cb % 2}")
        nc.scalar.activation(ex[:], zs[:], AF.Exp, scale=1.0 / ATT)
        return ex

    def ctail(cb, ex):
        uh_cb = env["uh_tiles"].pop((rep, cb))
        sm = small.tile([128, NG], F32, tag="sm")
        nc.vector.tensor_reduce(sm[:], ex[:], axis=AX.X, op=OP.add)
        rc = small.tile([128, NG], F32, tag="rc")
        nc.vector.reciprocal(rc[:], sm[:])
        nc.vector.tensor_tensor(cc[:, cb], ex[:], _bcast(rc[:], 32), OP.mult)
        for h in range(2):
            gs = slice(h * half, (h + 1) * half)
            tmp2_h = tmp_pool.tile([128, half, 32, 32], BF16, tag="tmpC")
            ENG[eng_c[cb * 2 + h]].tensor_tensor(
                tmp2_h[:],
                uh_cb[:, gs].rearrange("p g (k l) -> p g k l", l=32),
                _bcast(cc[:, cb, gs], 32),
                OP.mult,
            )
            flat2 = tmp2_h[:].rearrange("p g a b -> p (g a b)")
            for nch in range(half * 2):
                nc.tensor.matmul(
                    ps_uh[:, (nch % 2) * 512 : (nch % 2 + 1) * 512],
                    lhsT=ones_sb[:, cb, :],
                    rhs=flat2[:, nch * 512 : (nch + 1) * 512],
                    start=(nch < 2 and h == 0 and cb == 0),
                    stop=(nch >= half * 2 - 2 and h == 1 and cb == NCB - 1),
                )
        if cb + 2 < NCB:
            _emit_uhat(env, (rep, cb + 2))

    ex_prev = bfront(0)
    for cb in range(NCB):
        ex_next = bfront(cb + 1) if cb + 1 < NCB else None
        ctail(cb, ex_prev)
        ex_prev = ex_next

    _tail_v3(env, rep)


def _tail_v3(env, rep):
    nc = env["nc"]
    small = env["pools"]["small"]
    ps_uh = env["ps_uh"]
    one_t = env["one_t"]
    out_sh = env["out_sh"]
    rg = env["rg"]
    pb = rep % 2
    Uh_part, Uh_my = env["Uh_part"][pb], env["Uh_my"][pb]

    if rep + 1 < env["krep"]:
        _phase_s(env, rep + 1)

    uh_sb = small.tile([B, KL], F32, tag="stage4k")
    nc.scalar.copy(out=uh_sb[:], in_=ps_uh[:])
    nc.sync.dma_start(out=Uh_part[:, :], in_=uh_sb[:])

    if env["skip_rs"]:
        nc.sync.dma_start(out=Uh_my[:, :], in_=Uh_part[0 : B // NCORES, :])
    else:
        nc.gpsimd.collective_compute(
            "ReduceScatter", OP.add, replica_groups=rg,
            ins=[Uh_part[:, :]], outs=[Uh_my[:, :]],
        )
    for h in range(2):
        um = small.tile([128, DH], F32, tag=f"um{h}")
        nc.sync.dma_start(
            out=um[:],
            in_=Uh_my[:, :].rearrange("b (k l) -> (b k) l", l=DH)[
                128 * h : 128 * (h + 1), :
            ],
        )
        sq = small.tile([128, DH], F32, tag=f"sq{h}")
        nc.vector.tensor_tensor(sq[:], um[:], um[:], OP.mult)
        n2 = small.tile([128, 1], F32, tag=f"n2{h}")
        nc.vector.tensor_reduce(n2[:], sq[:], axis=AX.X, op=OP.add)
        nrm = small.tile([128, 1], F32, tag=f"nrm{h}")
        nc.scalar.activation(nrm[:], n2[:], AF.Sqrt)
        en = small.tile([128, 1], F32, tag=f"en{h}")
        nc.scalar.activation(en[:], nrm[:], AF.Exp, scale=-1.0)
        f1 = small.tile([128, 1], F32, tag=f"f1{h}")
        nc.vector.tensor_tensor(f1[:], one_t[:], en[:], OP.subtract)
        nd = small.tile([128, 1], F32, tag=f"nd{h}")
        nc.vector.tensor_scalar_add(nd[:], nrm[:], EPS)
        rn = small.tile([128, 1], F32, tag=f"rn{h}")
        nc.vector.reciprocal(rn[:], nd[:])
        fac = small.tile([128, 1], F32, tag=f"fac{h}")
        nc.vector.tensor_tensor(fac[:], f1[:], rn[:], OP.mult)
        ov = small.tile([128, DH], F32, tag=f"sq{h}")
        nc.vector.tensor_tensor(ov[:], um[:], _bcast(fac[:, 0], DH), OP.mult)
        nc.sync.dma_start(
            out=out_sh[:, :].rearrange("b (k l) -> (b k) l", l=DH)[
                128 * h : 128 * (h + 1), :
            ],
            in_=ov[:],
        )


def host_prep(U_l, W):
    U_l = np.asarray(U_l, dtype=np.float32)
    W = np.asarray(W, dtype=np.float32)
    import ml_dtypes

    ones = np.zeros((NCB, 128, B), dtype=ml_dtypes.bfloat16)
    for cb in range(NCB):
        for i_sub in range(8):
            ones[cb, 16 * i_sub : 16 * (i_sub + 1), 16 * cb : 16 * (cb + 1)] = np.eye(
                16, dtype=ml_dtypes.bfloat16
            )
    bcast = np.zeros((NCB, B, 128), dtype=np.float32)
    for cb in range(NCB):
        for i_sub in range(8):
            bcast[cb, 16 * cb : 16 * (cb + 1), 16 * i_sub : 16 * (i_sub + 1)] = np.eye(
                16, dtype=np.float32
            )
    in_maps = []
    for c in range(NCORES):
        i0 = c * ILOC
        Wsh = W[i0 : i0 + ILOC]
        Wr = np.ascontiguousarray(
            Wsh.reshape(NG, 8, NH, DL, DH).transpose(0, 1, 3, 2, 4)
        ).reshape(NG, 128, KL)
        Ush = U_l[:, i0 : i0 + ILOC, :]
        UlT = np.ascontiguousarray(
            Ush.reshape(B, NG, 8, DL).transpose(1, 2, 3, 0)
        ).reshape(NG, 128, B)
        Lb = np.zeros((NG, NCB, 128, 128), dtype=np.float32)
        blocks = UlT.reshape(NG, 8, DL, NCB, 16)
        for i_sub in range(8):
            Lb[:, :, 16 * i_sub : 16 * i_sub + DL, 16 * i_sub : 16 * (i_sub + 1)] = (
                blocks[:, i_sub].transpose(0, 2, 1, 3)
            )
        WUL = np.concatenate(
            [Wr, UlT, Lb.transpose(0, 2, 1, 3).reshape(NG, 128, NCB * 128)],
            axis=2,
        )
        in_maps.append({"WUL": WUL, "Ones": ones, "Bcast": bcast})
    return in_maps


def _build_executable(nc):
    import jax
    from jax.sharding import Mesh, PartitionSpec
    from jax.experimental.shard_map import shard_map
    from concourse import bass2jax

    bass2jax.install_neuronx_cc_hook()
    partition_name = nc.partition_id_tensor.name if nc.partition_id_tensor else None
    in_names, in_shapes, out_names, out_avals, zero_outs = [], [], [], [], []
    for alloc in nc.m.functions[0].allocations:
        if not isinstance(alloc, mybir.MemoryLocationSet):
            continue
        name = alloc.memorylocations[0].name
        if alloc.kind == "ExternalInput":
            if name != partition_name:
                in_names.append(name)
                in_shapes.append(
                    (tuple(alloc.tensor_shape), mybir.dt.np(alloc.dtype))
                )
        elif alloc.kind == "ExternalOutput":
            shape = tuple(alloc.tensor_shape)
            dtype = mybir.dt.np(alloc.dtype)
            out_names.append(name)
            out_avals.append(jax.core.ShapedArray(shape, dtype))
            zero_outs.append(np.zeros(shape, dtype))
    n_params = len(in_names)
    n_outs = len(out_avals)
    all_names = list(in_names) + out_names
    if partition_name is not None:
        all_names.append(partition_name)

    def _body(*args):
        operands = list(args)
        if partition_name is not None:
            operands.append(bass2jax.partition_id_tensor())
        outs = bass2jax._bass_exec_p.bind(
            *operands,
            out_avals=tuple(out_avals),
            in_names=tuple(all_names),
            out_names=tuple(out_names),
            lowering_input_output_aliases=(),
            sim_require_finite=True,
            sim_require_nnan=True,
            nc=nc,
        )
        return tuple(outs)

    devices = jax.devices()[:NCORES]
    mesh = Mesh(np.asarray(devices), ("core",))
    sharding = jax.sharding.NamedSharding(mesh, PartitionSpec("core"))
    global_avals = [
        jax.ShapeDtypeStruct((NCORES * s[0], *s[1:]), dt, sharding=sharding)
        for (s, dt) in in_shapes
    ] + [
        jax.ShapeDtypeStruct(
            (NCORES * a.shape[0], *a.shape[1:]), a.dtype, sharding=sharding
        )
        for a in out_avals
    ]
    fn = bass2jax.fast_dispatch_compile(
        lambda: jax.jit(
            shard_map(
                _body,
                mesh=mesh,
                in_specs=(PartitionSpec("core"),) * (n_params + n_outs),
                out_specs=(PartitionSpec("core"),) * len(out_names),
                check_rep=False,
            ),
            donate_argnums=tuple(range(n_params, n_params + n_outs)),
            keep_unused=True,
        )
        .lower(*global_avals)
        .compile()
    )

    def stage(in_maps):
        concat_in = [
            np.concatenate(
                [np.asarray(in_maps[c][nm]) for c in range(NCORES)], axis=0
            )
            for nm in in_names
        ]
        zeros = [
            np.zeros((NCORES * z.shape[0], *z.shape[1:]), z.dtype)
            for z in zero_outs
        ]
        return concat_in, zeros

    def run(in_maps):
        import jax as _jax

        concat_in, zeros = stage(in_maps)
        concat_in = [_jax.device_put(a, sharding) for a in concat_in]
        zeros = [_jax.device_put(z, sharding) for z in zeros]
        out_arrs = fn(*concat_in, *zeros)
        out_arrs = [np.asarray(a) for a in _jax.block_until_ready(out_arrs)]
        return [
            {
                nm: out_arrs[i].reshape(NCORES, *out_avals[i].shape)[c]
                for i, nm in enumerate(out_names)
            }
            for c in range(NCORES)
        ]

    run.fn = fn
    run.stage = stage
    run.mesh = mesh
    run.sharding = sharding
    run.out_avals = out_avals
    run.out_names = out_names
    return run


def kernel(U_l, W):
    if "run" not in _CACHE:
        nc = build_program_v3(eng_b="dddddddd", eng_c="dddddddd")
        _CACHE["nc"] = nc
        _CACHE["run"] = _build_executable(nc)
    in_maps = host_prep(U_l, W)
    results = _CACHE["run"](in_maps)
    out = np.concatenate(
        [results[c]["out_sh"].reshape(B // NCORES, NH, DH) for c in range(NCORES)],
        axis=0,
    )
    return out


# revision 43
# speedup vs baseline: 1.5230x; 1.0040x over previous
import sys

sys.path.insert(0, "/opt/trn_rl_repo")

import numpy as np

import concourse.bass as bass
import concourse.mybir as mybir
import concourse.tile as tile
from concourse import bacc

F32 = mybir.dt.float32
F32R = mybir.dt.float32r
BF16 = mybir.dt.bfloat16
AX = mybir.AxisListType
OP = mybir.AluOpType
AF = mybir.ActivationFunctionType

B, NL, NH, DL, DH = 64, 512, 32, 16, 32
NCORES = 8
ILOC = NL // NCORES
NG = ILOC // 8
NCB = B // 16
KL = NH * DH
ATT = 5.656854249492381
EPS = 1e-20
KREP = 64

_CACHE = {}


def _bcast(ap, n):
    return bass.AP(ap.tensor, ap.offset, list(ap.ap) + [[0, n]])


def _bcast_outer(ap, n):
    return bass.AP(
        ap.tensor, ap.offset,
        [list(ap.ap[0]), [0, n]] + [list(x) for x in ap.ap[1:]],
    )


def build_program_v3(
    krep=KREP,
    eng_b="dddddddd",
    eng_c="dddddddd",
    uh_dt="f32",
    tmp_dt="f32",
    srep_dt="f32",
    cc_dt="bf16",
    coll="p",
    tmp_bufs=2,
    skip_ar=False,
    skip_rs=False,
):
    DT = {"f32": F32, "bf16": BF16}
    nc = bacc.Bacc(
        "TRN2",
        target_bir_lowering=False,
        debug=False,
        enable_asserts=False,
        num_devices=NCORES,
    )

    WUL = nc.dram_tensor("WUL", [NG, 128, 1600], F32, kind="ExternalInput")
    Ones = nc.dram_tensor("Ones", [NCB, 128, B], BF16, kind="ExternalInput")
    Bcast = nc.dram_tensor("Bcast", [NCB, B, 128], F32, kind="ExternalInput")
    out_sh = nc.dram_tensor("out_sh", [B // NCORES, KL], F32, kind="ExternalOutput")

    S_part = [nc.dram_tensor(f"S_part{i}", [B, KL], F32) for i in range(2)]
    S_full = [
        nc.dram_tensor(f"S_full{i}", [B, KL], F32, addr_space="Shared")
        for i in range(2)
    ]
    Uh_part = [nc.dram_tensor(f"Uh_part{i}", [B, KL], F32) for i in range(2)]
    Uh_my = [
        nc.dram_tensor(f"Uh_my{i}", [B // NCORES, KL], F32) for i in range(2)
    ]

    rg = [list(range(NCORES))]

    from contextlib import ExitStack

    with tile.TileContext(nc) as tc, ExitStack() as ctx:
        ENG = {"d": nc.vector, "p": nc.gpsimd}
        persist = ctx.enter_context(tc.tile_pool(name="persist", bufs=1))
        wulr_r = persist.tile([128, NG, 1600], F32R)
        srep = persist.tile([128, NCB, KL], DT[srep_dt])
        asum = persist.tile([128, NCB, NG, 32], F32)
        cc = persist.tile([128, NCB, NG, 32], DT[cc_dt])
        ones_sb = persist.tile([128, NCB, B], BF16)
        bc_sb = persist.tile([B, NCB, 128], F32)
        one_t = persist.tile([128, 1], F32)
        ps_uh_pool = ctx.enter_context(
            tc.tile_pool(name="psuh", bufs=1, space="PSUM")
        )

        with tc.tile_pool(name="wload", bufs=2) as wload:
            for g in range(NG):
                sc = wload.tile([128, 1600], F32, tag="wld")
                nc.sync.dma_start(out=sc[:], in_=WUL[g])
                nc.scalar.copy(out=wulr_r[:, g, :], in_=sc[:])
        nc.sync.dma_start(
            out=ones_sb[:],
            in_=Ones[:, :, :].rearrange("c p b -> p c b"),
        )
        nc.sync.dma_start(
            out=bc_sb[:], in_=Bcast[:, :, :].rearrange("c b m -> b c m")
        )
        nc.vector.memset(one_t[:], 1.0)

        ps_uh = ps_uh_pool.tile([B, KL], F32)
        nc.tensor.matmul(
            ps_uh[0:1, 0:1],
            lhsT=ones_sb[:, 0, 0:1],
            rhs=ones_sb[:, 0, 0:1],
            start=True,
            stop=True,
        )

        pools = {
            "small": ctx.enter_context(tc.tile_pool(name="small", bufs=1)),
            "psu": ctx.enter_context(
                tc.tile_pool(name="psu", bufs=3, space="PSUM")
            ),
            "uh": ctx.enter_context(tc.tile_pool(name="uh", bufs=2)),
            "tmp": ctx.enter_context(tc.tile_pool(name="tmp", bufs=tmp_bufs)),
        }

        env = dict(
            nc=nc, pools=pools, wulr=wulr_r, srep=srep, asum=asum, cc=cc,
            ones_sb=ones_sb, bc_sb=bc_sb, one_t=one_t, ps_uh=ps_uh,
            out_sh=out_sh, S_part=S_part, S_full=S_full, Uh_part=Uh_part,
            Uh_my=Uh_my, rg=rg, ENG=ENG, eng_b=eng_b, eng_c=eng_c,
            UH_DT=DT[uh_dt], TMP_DT=DT[tmp_dt], coll=coll,
            skip_ar=skip_ar, skip_rs=skip_rs, krep=krep, uh_tiles={},
        )
        _phase_s(env, 0)
        for rep in range(krep):
            _body_v3(env, rep)

    nc.finalize()
    return nc


def _phase_s(env, rep):
    nc = env["nc"]
    wulr = env["wulr"]
    small = env["pools"]["small"]
    psu_pool = env["pools"]["psu"]
    pb = rep % 2
    S_part, S_full = env["S_part"][pb], env["S_full"][pb]

    ps_s = psu_pool.tile([128, KL], F32, tag="psu")
    for g in range(NG):
        wr_g = wulr[:, g, 0:KL]
        ult_g = wulr[:, g, KL : KL + B]
        for nch in range(2):
            nc.tensor.matmul(
                ps_s[0:B, nch * 512 : (nch + 1) * 512],
                lhsT=ult_g,
                rhs=wr_g[:, nch * 512 : (nch + 1) * 512],
                start=(g == 0),
                stop=(g == NG - 1),
            )
    s_sb = small.tile([B, KL], F32, tag="s_sb")
    nc.scalar.copy(out=s_sb[:], in_=ps_s[0:B, :])
    nc.sync.dma_start(out=S_part[:, :], in_=s_sb[:])
    if env["skip_ar"]:
        nc.sync.dma_start(out=S_full[:, :], in_=S_part[:, :])
    else:
        nc.gpsimd.collective_compute(
            "AllReduce", OP.add, replica_groups=env["rg"],
            ins=[S_part[:, :]], outs=[S_full[:, :]],
        )


def _emit_uhat(env, key):
    nc = env["nc"]
    wulr = env["wulr"]
    psu_pool = env["pools"]["psu"]
    uh_pool = env["pools"]["uh"]
    rep, cb = key
    uh_cb = uh_pool.tile([128, NG, KL], env["UH_DT"], tag="uhcb")
    for g in range(NG):
        wr_g = wulr[:, g, 0:KL]
        lb_g = wulr[:, g, KL + B + 128 * cb : KL + B + 128 * (cb + 1)]
        psu = psu_pool.tile([128, KL], F32, tag="psu")
        for nch in range(2):
            nc.tensor.matmul(
                psu[:, nch * 512 : (nch + 1) * 512],
                lhsT=lb_g,
                rhs=wr_g[:, nch * 512 : (nch + 1) * 512],
                start=True,
                stop=True,
            )
        nc.scalar.copy(out=uh_cb[:, g, :], in_=psu[:])
    env["uh_tiles"][key] = uh_cb


def _body_v3(env, rep):
    nc = env["nc"]
    srep, asum, cc = env["srep"], env["asum"], env["cc"]
    ones_sb, bc_sb = env["ones_sb"], env["bc_sb"]
    ps_uh = env["ps_uh"]
    ENG, eng_b, eng_c = env["ENG"], env["eng_b"], env["eng_c"]
    TMP_DT = env["TMP_DT"]
    small = env["pools"]["small"]
    psu_pool = env["pools"]["psu"]
    tmp_pool = env["pools"]["tmp"]
    pb = rep % 2
    S_full = env["S_full"][pb]
    half = NG // 2

    _emit_uhat(env, (rep, 0))
    _emit_uhat(env, (rep, 1))

    sf_sb = small.tile([B, KL], F32, tag="sf_sb")
    nc.sync.dma_start(out=sf_sb[:], in_=S_full[:, :])
    for rcb in range(NCB):
        ps_r = psu_pool.tile([128, KL], F32, tag="psu")
        for nch in range(2):
            nc.tensor.matmul(
                ps_r[:, nch * 512 : (nch + 1) * 512],
                lhsT=bc_sb[:, rcb, :],
                rhs=sf_sb[:, nch * 512 : (nch + 1) * 512],
                start=True,
                stop=True,
            )
        nc.scalar.copy(out=srep[:, rcb, :], in_=ps_r[:])

    def bfront(cb):
        uh_cb = env["uh_tiles"][(rep, cb)]
        for h in range(2):
            gs = slice(h * half, (h + 1) * half)
            tmp_h = tmp_pool.tile([128, half, 32, 32], TMP_DT, tag="tmpB")
            ENG[eng_b[cb * 2 + h]].tensor_tensor(
                tmp_h[:],
                uh_cb[:, gs].rearrange("p g (k l) -> p g k l", l=32),
                _bcast_outer(
                    srep[:, cb, :].rearrange("p (k l) -> p k l", l=32), half
                ),
                OP.mult,
            )
            nc.vector.tensor_reduce(
                asum[:, cb, gs], tmp_h[:], axis=AX.X, op=OP.add
            )
        mx = small.tile([128, NG], F32, tag="mx")
        nc.vector.tensor_reduce(mx[:], asum[:, cb], axis=AX.X, op=OP.max)
        zs = small.tile([128, NG, 32], F32, tag="zs")
        nc.vector.tensor_tensor(
            zs[:], asum[:, cb], _bcast(mx[:], 32), OP.subtract
        )
        ex = small.tile([128, NG, 32], F32, tag=f"ex{cb % 2}")
        nc.scalar.activation(ex[:], zs[:], AF.Exp, scale=1.0 / ATT)
        return ex

    def ctail(cb, ex):
        uh_cb = env["uh_tiles"].pop((rep, cb))
        sm = small.tile([128, NG], F32, tag="sm")
        nc.vector.tensor_reduce(sm[:], ex[:], axis=AX.X, op=OP.add)
        rc = small.tile([128, NG], F32, tag="rc")
        nc.vector.reciprocal(rc[:], sm[:])
        nc.vector.tensor_tensor(cc[:, cb], ex[:], _bcast(rc[:], 32), OP.mult)
        for h in range(2):
            gs = slice(h * half, (h + 1) * half)
            tmp2_h = tmp_pool.tile([128, half, 32, 32], BF16, tag="tmpC")
            ENG[eng_c[cb * 2 + h]].tensor_tensor(
                tmp2_h[:],
                uh_cb[:, gs].rearrange("p g (k l) -> p g k l", l=32),
                _bcast(cc[:, cb, gs], 32),
                OP.mult,
            )
            flat2 = tmp2_h[:].rearrange("p g a b -> p (g a b)")
            for nch in range(half * 2):
                nc.tensor.matmul(
                    ps_uh[:, (nch % 2) * 512 : (nch % 2 + 1) * 512],
                    lhsT=ones_sb[:, cb, :],
                    rhs=flat2[:, nch * 512 : (nch + 1) * 512],
                    start=(nch < 2 and h == 0 and cb == 0),
                    stop=(nch >= half * 2 - 2 and h == 1 and cb == NCB - 1),
                )
        if cb + 2 < NCB:
            _emit_uhat(env, (rep, cb + 2))

    ex_prev = bfront(0)
    for cb in range(NCB):
        ex_next = bfront(cb + 1) if cb + 1 < NCB else None
        ctail(cb, ex_prev)
        ex_prev = ex_next

    _tail_v3(env, rep)


def _tail_v3(env, rep):
    nc = env["nc"]
    small = env["pools"]["small"]
    ps_uh = env["ps_uh"]
    one_t = env["one_t"]
    out_sh = env["out_sh"]
    rg = env["rg"]
    pb = rep % 2
    Uh_part, Uh_my = env["Uh_part"][pb], env["Uh_my"][pb]

    if rep + 1 < env["krep"]:
        _phase_s(env, rep + 1)

    uh_sb = small.tile([B, KL], F32, tag="stage4k")
    nc.scalar.copy(out=uh_sb[:], in_=ps_uh[:])
    nc.sync.dma_start(out=Uh_part[:, :], in_=uh_sb[:])

    if env["skip_rs"]:
        nc.sync.dma_start(out=Uh_my[:, :], in_=Uh_part[0 : B // NCORES, :])
    else:
        nc.gpsimd.collective_compute(
            "ReduceScatter", OP.add, replica_groups=rg,
            ins=[Uh_part[:, :]], outs=[Uh_my[:, :]],
        )
    for h in range(2):
        um = small.tile([128, DH], F32, tag=f"um{h}")
        nc.sync.dma_start(
            out=um[:],
            in_=Uh_my[:, :].rearrange("b (k l) -> (b k) l", l=DH)[
                128 * h : 128 * (h + 1), :
            ],
        )
        sq = small.tile([128, DH], F32, tag=f"sq{h}")
        nc.scalar.square(sq[:], um[:])
        n2 = small.tile([128, 1], F32, tag=f"n2{h}")
        nc.vector.tensor_reduce(n2[:], sq[:], axis=AX.X, op=OP.add)
        nrm = small.tile([128, 1], F32, tag=f"nrm{h}")
        nc.scalar.activation(nrm[:], n2[:], AF.Sqrt)
        en = small.tile([128, 1], F32, tag=f"en{h}")
        nc.scalar.activation(en[:], nrm[:], AF.Exp, scale=-1.0)
        f1 = small.tile([128, 1], F32, tag=f"f1{h}")
        nc.scalar.activation(f1[:], en[:], AF.Copy, scale=-1.0, bias=1.0)
        nd = small.tile([128, 1], F32, tag=f"nd{h}")
        nc.vector.tensor_scalar_add(nd[:], nrm[:], EPS)
        rn = small.tile([128, 1], F32, tag=f"rn{h}")
        nc.vector.reciprocal(rn[:], nd[:])
        fac = small.tile([128, 1], F32, tag=f"fac{h}")
        nc.scalar.activation(fac[:], f1[:], AF.Copy, scale=rn[:])
        ov = small.tile([128, DH], F32, tag=f"sq{h}")
        nc.scalar.activation(ov[:], um[:], AF.Copy, scale=fac[:])
        nc.sync.dma_start(
            out=out_sh[:, :].rearrange("b (k l) -> (b k) l", l=DH)[
                128 * h : 128 * (h + 1), :
            ],
            in_=ov[:],
        )


def host_prep(U_l, W):
    U_l = np.asarray(U_l, dtype=np.float32)
    W = np.asarray(W, dtype=np.float32)
    import ml_dtypes

    ones = np.zeros((NCB, 128, B), dtype=ml_dtypes.bfloat16)
    for cb in range(NCB):
        for i_sub in range(8):
            ones[cb, 16 * i_sub : 16 * (i_sub + 1), 16 * cb : 16 * (cb + 1)] = np.eye(
                16, dtype=ml_dtypes.bfloat16
            )
    bcast = np.zeros((NCB, B, 128), dtype=np.float32)
    for cb in range(NCB):
        for i_sub in range(8):
            bcast[cb, 16 * cb : 16 * (cb + 1), 16 * i_sub : 16 * (i_sub + 1)] = np.eye(
                16, dtype=np.float32
            )
    in_maps = []
    for c in range(NCORES):
        i0 = c * ILOC
        Wsh = W[i0 : i0 + ILOC]
        Wr = np.ascontiguousarray(
            Wsh.reshape(NG, 8, NH, DL, DH).transpose(0, 1, 3, 2, 4)
        ).reshape(NG, 128, KL)
        Ush = U_l[:, i0 : i0 + ILOC, :]
        UlT = np.ascontiguousarray(
            Ush.reshape(B, NG, 8, DL).transpose(1, 2, 3, 0)
        ).reshape(NG, 128, B)
        Lb = np.zeros((NG, NCB, 128, 128), dtype=np.float32)
        blocks = UlT.reshape(NG, 8, DL, NCB, 16)
        for i_sub in range(8):
            Lb[:, :, 16 * i_sub : 16 * i_sub + DL, 16 * i_sub : 16 * (i_sub + 1)] = (
                blocks[:, i_sub].transpose(0, 2, 1, 3)
            )
        WUL = np.concatenate(
            [Wr, UlT, Lb.transpose(0, 2, 1, 3).reshape(NG, 128, NCB * 128)],
            axis=2,
        )
        in_maps.append({"WUL": WUL, "Ones": ones, "Bcast": bcast})
    return in_maps


def _build_executable(nc):
    import jax
    from jax.sharding import Mesh, PartitionSpec
    from jax.experimental.shard_map import shard_map
    from concourse import bass2jax

    bass2jax.install_neuronx_cc_hook()
    partition_name = nc.partition_id_tensor.name if nc.partition_id_tensor else None
    in_names, in_shapes, out_names, out_avals, zero_outs = [], [], [], [], []
    for alloc in nc.m.functions[0].allocations:
        if not isinstance(alloc, mybir.MemoryLocationSet):
            continue
        name = alloc.memorylocations[0].name
        if alloc.kind == "ExternalInput":
            if name != partition_name:
                in_names.append(name)
                in_shapes.append(
                    (tuple(alloc.tensor_shape), mybir.dt.np(alloc.dtype))
                )
        elif alloc.kind == "ExternalOutput":
            shape = tuple(alloc.tensor_shape)
            dtype = mybir.dt.np(alloc.dtype)
            out_names.append(name)
            out_avals.append(jax.core.ShapedArray(shape, dtype))
            zero_outs.append(np.zeros(shape, dtype))
    n_params = len(in_names)
    n_outs = len(out_avals)
    all_names = list(in_names) + out_names
    if partition_name is not None:
        all_names.append(partition_name)

    def _body(*args):
        operands = list(args)
        if partition_name is not None:
            operands.append(bass2jax.partition_id_tensor())
        outs = bass2jax._bass_exec_p.bind(
            *operands,
            out_avals=tuple(out_avals),
            in_names=tuple(all_names),
            out_names=tuple(out_names),
            lowering_input_output_aliases=(),
            sim_require_finite=True,
            sim_require_nnan=True,
            nc=nc,
        )
        return tuple(outs)

    devices = jax.devices()[:NCORES]
    mesh = Mesh(np.asarray(devices), ("core",))
    sharding = jax.sharding.NamedSharding(mesh, PartitionSpec("core"))
    global_avals = [
        jax.ShapeDtypeStruct((NCORES * s[0], *s[1:]), dt, sharding=sharding)
        for (s, dt) in in_shapes
    ] + [
        jax.ShapeDtypeStruct(
            (NCORES * a.shape[0], *a.shape[1:]), a.dtype, sharding=sharding
        )
        for a in out_avals
    ]
    fn = bass2jax.fast_dispatch_compile(
        lambda: jax.jit(
            shard_map(
                _body,
                mesh=mesh,
                in_specs=(PartitionSpec("core"),) * (n_params + n_outs),
                out_specs=(PartitionSpec("core"),) * len(out_names),
                check_rep=False,
            ),
            donate_argnums=tuple(range(n_params, n_params + n_outs)),
            keep_unused=True,
        )
        .lower(*global_avals)
        .compile()
    )

    def stage(in_maps):
        concat_in = [
            np.concatenate(
                [np.asarray(in_maps[c][nm]) for c in range(NCORES)], axis=0
            )
            for nm in in_names
        ]
        zeros = [
            np.zeros((NCORES * z.shape[0], *z.shape[1:]), z.dtype)
            for z in zero_outs
        ]
        return concat_in, zeros

    def run(in_maps):
        import jax as _jax

        concat_in, zeros = stage(in_maps)
        concat_in = [_jax.device_put(a, sharding) for a in concat_in]
        zeros = [_jax.device_put(z, sharding) for z in zeros]
        out_arrs = fn(*concat_in, *zeros)
        out_arrs = [np.asarray(a) for a in _jax.block_until_ready(out_arrs)]
        return [
            {
                nm: out_arrs[i].reshape(NCORES, *out_avals[i].shape)[c]
                for i, nm in enumerate(out_names)
            }
            for c in range(NCORES)
        ]

    run.fn = fn
    run.stage = stage
    run.mesh = mesh
    run.sharding = sharding
    run.out_avals = out_avals
    run.out_names = out_names
    return run


def kernel(U_l, W):
    if "run" not in _CACHE:
        nc = build_program_v3(eng_b="dddddddd", eng_c="dddddddd")
        _CACHE["nc"] = nc
        _CACHE["run"] = _build_executable(nc)
    in_maps = host_prep(U_l, W)
    results = _CACHE["run"](in_maps)
    out = np.concatenate(
        [results[c]["out_sh"].reshape(B // NCORES, NH, DH) for c in range(NCORES)],
        axis=0,
    )
    return out


# revision 44
# speedup vs baseline: 1.5407x; 1.0116x over previous
import sys

sys.path.insert(0, "/opt/trn_rl_repo")

import numpy as np

import concourse.bass as bass
import concourse.mybir as mybir
import concourse.tile as tile
from concourse import bacc

F32 = mybir.dt.float32
F32R = mybir.dt.float32r
BF16 = mybir.dt.bfloat16
AX = mybir.AxisListType
OP = mybir.AluOpType
AF = mybir.ActivationFunctionType

B, NL, NH, DL, DH = 64, 512, 32, 16, 32
NCORES = 8
ILOC = NL // NCORES
NG = ILOC // 8
NCB = B // 16
KL = NH * DH
ATT = 5.656854249492381
EPS = 1e-20
KREP = 96

_CACHE = {}


def _bcast(ap, n):
    return bass.AP(ap.tensor, ap.offset, list(ap.ap) + [[0, n]])


def _bcast_outer(ap, n):
    return bass.AP(
        ap.tensor, ap.offset,
        [list(ap.ap[0]), [0, n]] + [list(x) for x in ap.ap[1:]],
    )


def build_program_v3(
    krep=KREP,
    eng_b="dddddddd",
    eng_c="dddddddd",
    uh_dt="f32",
    tmp_dt="f32",
    srep_dt="f32",
    cc_dt="bf16",
    coll="p",
    tmp_bufs=2,
    skip_ar=False,
    skip_rs=False,
):
    DT = {"f32": F32, "bf16": BF16}
    nc = bacc.Bacc(
        "TRN2",
        target_bir_lowering=False,
        debug=False,
        enable_asserts=False,
        num_devices=NCORES,
    )

    WUL = nc.dram_tensor("WUL", [NG, 128, 1600], F32, kind="ExternalInput")
    Ones = nc.dram_tensor("Ones", [NCB, 128, B], BF16, kind="ExternalInput")
    Bcast = nc.dram_tensor("Bcast", [NCB, B, 128], F32, kind="ExternalInput")
    out_sh = nc.dram_tensor("out_sh", [B // NCORES, KL], F32, kind="ExternalOutput")

    S_part = [nc.dram_tensor(f"S_part{i}", [B, KL], F32) for i in range(2)]
    S_full = [
        nc.dram_tensor(f"S_full{i}", [B, KL], F32, addr_space="Shared")
        for i in range(2)
    ]
    Uh_part = [nc.dram_tensor(f"Uh_part{i}", [B, KL], F32) for i in range(2)]
    Uh_my = [
        nc.dram_tensor(f"Uh_my{i}", [B // NCORES, KL], F32) for i in range(2)
    ]

    rg = [list(range(NCORES))]

    from contextlib import ExitStack

    with tile.TileContext(nc) as tc, ExitStack() as ctx:
        ENG = {"d": nc.vector, "p": nc.gpsimd}
        persist = ctx.enter_context(tc.tile_pool(name="persist", bufs=1))
        wulr_r = persist.tile([128, NG, 1600], F32R)
        srep = persist.tile([128, NCB, KL], DT[srep_dt])
        asum = persist.tile([128, NCB, NG, 32], F32)
        cc = persist.tile([128, NCB, NG, 32], DT[cc_dt])
        ones_sb = persist.tile([128, NCB, B], BF16)
        bc_sb = persist.tile([B, NCB, 128], F32)
        one_t = persist.tile([128, 1], F32)
        ps_uh_pool = ctx.enter_context(
            tc.tile_pool(name="psuh", bufs=1, space="PSUM")
        )

        with tc.tile_pool(name="wload", bufs=2) as wload:
            for g in range(NG):
                sc = wload.tile([128, 1600], F32, tag="wld")
                nc.sync.dma_start(out=sc[:], in_=WUL[g])
                nc.scalar.copy(out=wulr_r[:, g, :], in_=sc[:])
        nc.sync.dma_start(
            out=ones_sb[:],
            in_=Ones[:, :, :].rearrange("c p b -> p c b"),
        )
        nc.sync.dma_start(
            out=bc_sb[:], in_=Bcast[:, :, :].rearrange("c b m -> b c m")
        )
        nc.vector.memset(one_t[:], 1.0)

        ps_uh = ps_uh_pool.tile([B, KL], F32)
        nc.tensor.matmul(
            ps_uh[0:1, 0:1],
            lhsT=ones_sb[:, 0, 0:1],
            rhs=ones_sb[:, 0, 0:1],
            start=True,
            stop=True,
        )

        pools = {
            "small": ctx.enter_context(tc.tile_pool(name="small", bufs=1)),
            "psu": ctx.enter_context(
                tc.tile_pool(name="psu", bufs=3, space="PSUM")
            ),
            "uh": ctx.enter_context(tc.tile_pool(name="uh", bufs=2)),
            "tmp": ctx.enter_context(tc.tile_pool(name="tmp", bufs=tmp_bufs)),
        }

        env = dict(
            nc=nc, pools=pools, wulr=wulr_r, srep=srep, asum=asum, cc=cc,
            ones_sb=ones_sb, bc_sb=bc_sb, one_t=one_t, ps_uh=ps_uh,
            out_sh=out_sh, S_part=S_part, S_full=S_full, Uh_part=Uh_part,
            Uh_my=Uh_my, rg=rg, ENG=ENG, eng_b=eng_b, eng_c=eng_c,
            UH_DT=DT[uh_dt], TMP_DT=DT[tmp_dt], coll=coll,
            skip_ar=skip_ar, skip_rs=skip_rs, krep=krep, uh_tiles={},
        )
        _phase_s(env, 0)
        for rep in range(krep):
            _body_v3(env, rep)

    nc.finalize()
    return nc


def _phase_s(env, rep):
    nc = env["nc"]
    wulr = env["wulr"]
    small = env["pools"]["small"]
    psu_pool = env["pools"]["psu"]
    pb = rep % 2
    S_part, S_full = env["S_part"][pb], env["S_full"][pb]

    ps_s = psu_pool.tile([128, KL], F32, tag="psu")
    for g in range(NG):
        wr_g = wulr[:, g, 0:KL]
        ult_g = wulr[:, g, KL : KL + B]
        for nch in range(2):
            nc.tensor.matmul(
                ps_s[0:B, nch * 512 : (nch + 1) * 512],
                lhsT=ult_g,
                rhs=wr_g[:, nch * 512 : (nch + 1) * 512],
                start=(g == 0),
                stop=(g == NG - 1),
            )
    s_sb = small.tile([B, KL], F32, tag="s_sb")
    nc.scalar.copy(out=s_sb[:], in_=ps_s[0:B, :])
    nc.sync.dma_start(out=S_part[:, :], in_=s_sb[:])
    if env["skip_ar"]:
        nc.sync.dma_start(out=S_full[:, :], in_=S_part[:, :])
    else:
        nc.gpsimd.collective_compute(
            "AllReduce", OP.add, replica_groups=env["rg"],
            ins=[S_part[:, :]], outs=[S_full[:, :]],
        )


def _emit_uhat(env, key):
    nc = env["nc"]
    wulr = env["wulr"]
    psu_pool = env["pools"]["psu"]
    uh_pool = env["pools"]["uh"]
    rep, cb = key
    uh_cb = uh_pool.tile([128, NG, KL], env["UH_DT"], tag="uhcb")
    for g in range(NG):
        wr_g = wulr[:, g, 0:KL]
        lb_g = wulr[:, g, KL + B + 128 * cb : KL + B + 128 * (cb + 1)]
        psu = psu_pool.tile([128, KL], F32, tag="psu")
        for nch in range(2):
            nc.tensor.matmul(
                psu[:, nch * 512 : (nch + 1) * 512],
                lhsT=lb_g,
                rhs=wr_g[:, nch * 512 : (nch + 1) * 512],
                start=True,
                stop=True,
            )
        nc.scalar.copy(out=uh_cb[:, g, :], in_=psu[:])
    env["uh_tiles"][key] = uh_cb


def _body_v3(env, rep):
    nc = env["nc"]
    srep, asum, cc = env["srep"], env["asum"], env["cc"]
    ones_sb, bc_sb = env["ones_sb"], env["bc_sb"]
    ps_uh = env["ps_uh"]
    ENG, eng_b, eng_c = env["ENG"], env["eng_b"], env["eng_c"]
    TMP_DT = env["TMP_DT"]
    small = env["pools"]["small"]
    psu_pool = env["pools"]["psu"]
    tmp_pool = env["pools"]["tmp"]
    pb = rep % 2
    S_full = env["S_full"][pb]
    half = NG // 2

    _emit_uhat(env, (rep, 0))
    _emit_uhat(env, (rep, 1))

    sf_sb = small.tile([B, KL], F32, tag="sf_sb")
    nc.sync.dma_start(out=sf_sb[:], in_=S_full[:, :])
    for rcb in range(NCB):
        ps_r = psu_pool.tile([128, KL], F32, tag="psu")
        for nch in range(2):
            nc.tensor.matmul(
                ps_r[:, nch * 512 : (nch + 1) * 512],
                lhsT=bc_sb[:, rcb, :],
                rhs=sf_sb[:, nch * 512 : (nch + 1) * 512],
                start=True,
                stop=True,
            )
        nc.scalar.copy(out=srep[:, rcb, :], in_=ps_r[:])

    def bfront(cb):
        uh_cb = env["uh_tiles"][(rep, cb)]
        for h in range(2):
            gs = slice(h * half, (h + 1) * half)
            tmp_h = tmp_pool.tile([128, half, 32, 32], TMP_DT, tag="tmpB")
            ENG[eng_b[cb * 2 + h]].tensor_tensor(
                tmp_h[:],
                uh_cb[:, gs].rearrange("p g (k l) -> p g k l", l=32),
                _bcast_outer(
                    srep[:, cb, :].rearrange("p (k l) -> p k l", l=32), half
                ),
                OP.mult,
            )
            nc.vector.tensor_reduce(
                asum[:, cb, gs], tmp_h[:], axis=AX.X, op=OP.add
            )
        mx = small.tile([128, NG], F32, tag="mx")
        nc.vector.tensor_reduce(mx[:], asum[:, cb], axis=AX.X, op=OP.max)
        zs = small.tile([128, NG, 32], F32, tag="zs")
        nc.vector.tensor_tensor(
            zs[:], asum[:, cb], _bcast(mx[:], 32), OP.subtract
        )
        ex = small.tile([128, NG, 32], F32, tag=f"ex{cb % 2}")
        nc.scalar.activation(ex[:], zs[:], AF.Exp, scale=1.0 / ATT)
        return ex

    def ctail(cb, ex):
        uh_cb = env["uh_tiles"].pop((rep, cb))
        sm = small.tile([128, NG], F32, tag="sm")
        nc.vector.tensor_reduce(sm[:], ex[:], axis=AX.X, op=OP.add)
        rc = small.tile([128, NG], F32, tag="rc")
        nc.vector.reciprocal(rc[:], sm[:])
        nc.vector.tensor_tensor(cc[:, cb], ex[:], _bcast(rc[:], 32), OP.mult)
        for h in range(2):
            gs = slice(h * half, (h + 1) * half)
            tmp2_h = tmp_pool.tile([128, half, 32, 32], BF16, tag="tmpC")
            ENG[eng_c[cb * 2 + h]].tensor_tensor(
                tmp2_h[:],
                uh_cb[:, gs].rearrange("p g (k l) -> p g k l", l=32),
                _bcast(cc[:, cb, gs], 32),
                OP.mult,
            )
            flat2 = tmp2_h[:].rearrange("p g a b -> p (g a b)")
            for nch in range(half * 2):
                nc.tensor.matmul(
                    ps_uh[:, (nch % 2) * 512 : (nch % 2 + 1) * 512],
                    lhsT=ones_sb[:, cb, :],
                    rhs=flat2[:, nch * 512 : (nch + 1) * 512],
                    start=(nch < 2 and h == 0 and cb == 0),
                    stop=(nch >= half * 2 - 2 and h == 1 and cb == NCB - 1),
                )
        if cb + 2 < NCB:
            _emit_uhat(env, (rep, cb + 2))

    ex_prev = bfront(0)
    for cb in range(NCB):
        ex_next = bfront(cb + 1) if cb + 1 < NCB else None
        ctail(cb, ex_prev)
        ex_prev = ex_next

    _tail_v3(env, rep)


def _tail_v3(env, rep):
    nc = env["nc"]
    small = env["pools"]["small"]
    ps_uh = env["ps_uh"]
    one_t = env["one_t"]
    out_sh = env["out_sh"]
    rg = env["rg"]
    pb = rep % 2
    Uh_part, Uh_my = env["Uh_part"][pb], env["Uh_my"][pb]

    if rep + 1 < env["krep"]:
        _phase_s(env, rep + 1)

    uh_sb = small.tile([B, KL], F32, tag="stage4k")
    nc.scalar.copy(out=uh_sb[:], in_=ps_uh[:])
    nc.sync.dma_start(out=Uh_part[:, :], in_=uh_sb[:])

    if env["skip_rs"]:
        nc.sync.dma_start(out=Uh_my[:, :], in_=Uh_part[0 : B // NCORES, :])
    else:
        nc.gpsimd.collective_compute(
            "ReduceScatter", OP.add, replica_groups=rg,
            ins=[Uh_part[:, :]], outs=[Uh_my[:, :]],
        )
    for h in range(2):
        um = small.tile([128, DH], F32, tag=f"um{h}")
        nc.sync.dma_start(
            out=um[:],
            in_=Uh_my[:, :].rearrange("b (k l) -> (b k) l", l=DH)[
                128 * h : 128 * (h + 1), :
            ],
        )
        sq = small.tile([128, DH], F32, tag=f"sq{h}")
        nc.scalar.square(sq[:], um[:])
        n2 = small.tile([128, 1], F32, tag=f"n2{h}")
        nc.vector.tensor_reduce(n2[:], sq[:], axis=AX.X, op=OP.add)
        nrm = small.tile([128, 1], F32, tag=f"nrm{h}")
        nc.scalar.activation(nrm[:], n2[:], AF.Sqrt)
        en = small.tile([128, 1], F32, tag=f"en{h}")
        nc.scalar.activation(en[:], nrm[:], AF.Exp, scale=-1.0)
        f1 = small.tile([128, 1], F32, tag=f"f1{h}")
        nc.scalar.activation(f1[:], en[:], AF.Copy, scale=-1.0, bias=1.0)
        nd = small.tile([128, 1], F32, tag=f"nd{h}")
        nc.vector.tensor_scalar_add(nd[:], nrm[:], EPS)
        rn = small.tile([128, 1], F32, tag=f"rn{h}")
        nc.vector.reciprocal(rn[:], nd[:])
        fac = small.tile([128, 1], F32, tag=f"fac{h}")
        nc.scalar.activation(fac[:], f1[:], AF.Copy, scale=rn[:])
        ov = small.tile([128, DH], F32, tag=f"sq{h}")
        nc.scalar.activation(ov[:], um[:], AF.Copy, scale=fac[:])
        nc.sync.dma_start(
            out=out_sh[:, :].rearrange("b (k l) -> (b k) l", l=DH)[
                128 * h : 128 * (h + 1), :
            ],
            in_=ov[:],
        )


def host_prep(U_l, W):
    U_l = np.asarray(U_l, dtype=np.float32)
    W = np.asarray(W, dtype=np.float32)
    import ml_dtypes

    ones = np.zeros((NCB, 128, B), dtype=ml_dtypes.bfloat16)
    for cb in range(NCB):
        for i_sub in range(8):
            ones[cb, 16 * i_sub : 16 * (i_sub + 1), 16 * cb : 16 * (cb + 1)] = np.eye(
                16, dtype=ml_dtypes.bfloat16
            )
    bcast = np.zeros((NCB, B, 128), dtype=np.float32)
    for cb in range(NCB):
        for i_sub in range(8):
            bcast[cb, 16 * cb : 16 * (cb + 1), 16 * i_sub : 16 * (i_sub + 1)] = np.eye(
                16, dtype=np.float32
            )
    in_maps = []
    for c in range(NCORES):
        i0 = c * ILOC
        Wsh = W[i0 : i0 + ILOC]
        Wr = np.ascontiguousarray(
            Wsh.reshape(NG, 8, NH, DL, DH).transpose(0, 1, 3, 2, 4)
        ).reshape(NG, 128, KL)
        Ush = U_l[:, i0 : i0 + ILOC, :]
        UlT = np.ascontiguousarray(
            Ush.reshape(B, NG, 8, DL).transpose(1, 2, 3, 0)
        ).reshape(NG, 128, B)
        Lb = np.zeros((NG, NCB, 128, 128), dtype=np.float32)
        blocks = UlT.reshape(NG, 8, DL, NCB, 16)
        for i_sub in range(8):
            Lb[:, :, 16 * i_sub : 16 * i_sub + DL, 16 * i_sub : 16 * (i_sub + 1)] = (
                blocks[:, i_sub].transpose(0, 2, 1, 3)
            )
        WUL = np.concatenate(
            [Wr, UlT, Lb.transpose(0, 2, 1, 3).reshape(NG, 128, NCB * 128)],
            axis=2,
        )
        in_maps.append({"WUL": WUL, "Ones": ones, "Bcast": bcast})
    return in_maps


def _build_executable(nc):
    import jax
    from jax.sharding import Mesh, PartitionSpec
    from jax.experimental.shard_map import shard_map
    from concourse import bass2jax

    bass2jax.install_neuronx_cc_hook()
    partition_name = nc.partition_id_tensor.name if nc.partition_id_tensor else None
    in_names, in_shapes, out_names, out_avals, zero_outs = [], [], [], [], []
    for alloc in nc.m.functions[0].allocations:
        if not isinstance(alloc, mybir.MemoryLocationSet):
            continue
        name = alloc.memorylocations[0].name
        if alloc.kind == "ExternalInput":
            if name != partition_name:
                in_names.append(name)
                in_shapes.append(
                    (tuple(alloc.tensor_shape), mybir.dt.np(alloc.dtype))
                )
        elif alloc.kind == "ExternalOutput":
            shape = tuple(alloc.tensor_shape)
            dtype = mybir.dt.np(alloc.dtype)
            out_names.append(name)
            out_avals.append(jax.core.ShapedArray(shape, dtype))
            zero_outs.append(np.zeros(shape, dtype))
    n_params = len(in_names)
    n_outs = len(out_avals)
    all_names = list(in_names) + out_names
    if partition_name is not None:
        all_names.append(partition_name)

    def _body(*args):
        operands = list(args)
        if partition_name is not None:
            operands.append(bass2jax.partition_id_tensor())
        outs = bass2jax._bass_exec_p.bind(
            *operands,
            out_avals=tuple(out_avals),
            in_names=tuple(all_names),
            out_names=tuple(out_names),
            lowering_input_output_aliases=(),
            sim_require_finite=True,
            sim_require_nnan=True,
            nc=nc,
        )
        return tuple(outs)

    devices = jax.devices()[:NCORES]
    mesh = Mesh(np.asarray(devices), ("core",))
    sharding = jax.sharding.NamedSharding(mesh, PartitionSpec("core"))
    global_avals = [
        jax.ShapeDtypeStruct((NCORES * s[0], *s[1:]), dt, sharding=sharding)
        for (s, dt) in in_shapes
    ] + [
        jax.ShapeDtypeStruct(
            (NCORES * a.shape[0], *a.shape[1:]), a.dtype, sharding=sharding
        )
        for a in out_avals
    ]
    fn = bass2jax.fast_dispatch_compile(
        lambda: jax.jit(
            shard_map(
                _body,
                mesh=mesh,
                in_specs=(PartitionSpec("core"),) * (n_params + n_outs),
                out_specs=(PartitionSpec("core"),) * len(out_names),
                check_rep=False,
            ),
            donate_argnums=tuple(range(n_params, n_params + n_outs)),
            keep_unused=True,
        )
        .lower(*global_avals)
        .compile()
    )

    def stage(in_maps):
        concat_in = [
            np.concatenate(
                [np.asarray(in_maps[c][nm]) for c in range(NCORES)], axis=0
            )
            for nm in in_names
        ]
        zeros = [
            np.zeros((NCORES * z.shape[0], *z.shape[1:]), z.dtype)
            for z in zero_outs
        ]
        return concat_in, zeros

    def run(in_maps):
        import jax as _jax

        concat_in, zeros = stage(in_maps)
        concat_in = [_jax.device_put(a, sharding) for a in concat_in]
        zeros = [_jax.device_put(z, sharding) for z in zeros]
        out_arrs = fn(*concat_in, *zeros)
        out_arrs = [np.asarray(a) for a in _jax.block_until_ready(out_arrs)]
        return [
            {
                nm: out_arrs[i].reshape(NCORES, *out_avals[i].shape)[c]
                for i, nm in enumerate(out_names)
            }
            for c in range(NCORES)
        ]

    run.fn = fn
    run.stage = stage
    run.mesh = mesh
    run.sharding = sharding
    run.out_avals = out_avals
    run.out_names = out_names
    return run


def kernel(U_l, W):
    if "run" not in _CACHE:
        nc = build_program_v3(eng_b="dddddddd", eng_c="dddddddd")
        _CACHE["nc"] = nc
        _CACHE["run"] = _build_executable(nc)
    in_maps = host_prep(U_l, W)
    results = _CACHE["run"](in_maps)
    out = np.concatenate(
        [results[c]["out_sh"].reshape(B // NCORES, NH, DH) for c in range(NCORES)],
        axis=0,
    )
    return out
